# revision 23
# baseline (speedup 1.0000x reference)
"""Causal self-attention (B=4, T=2048, C=1024, H=16, D=64) on 8 trn2 cores.

Sharding: core i handles batch b = i//2 and head-group g = i%2 (8 of 16
heads), tensor-parallel over c_attn columns / c_proj rows. Each core
computes qkv for its heads, causal attention, and a partial projection
(its 512 rows of w_proj); the host sums the two partials per batch and
adds b_proj.

v2 pipeline (per core), built around the engine cost model:
  - q/k/v projections run on the PE in fp8 DoubleRow mode with a hi/lo
    split (x = x_hi + x_lo, w = w_hi + w_lo, three cross terms): 256-wide
    contraction per instruction at 0.5 cyc/col = 2.67x bf16 throughput
    at ~bf16 accuracy.
  - q/k are stored to SBUF as fp8 (e4m3) in a [64d, 2, T] pair-plane
    layout whose second plane is zeroed; S^T strips then also run in
    DoubleRow mode (2 cyc per 4 cols) despite the 64-deep contraction.
  - exp on ACT (the critical engine: ~139k cols x 0.83ns) writes bf16
    es strips; the causal diagonal block is masked by a DVE multiply.
  - AV is token-major: es strip blocks [128j, 128i] are the *stationary*
    operand, v_aug [128j, 65] (ones column -> denominators) the moving
    one, so each block costs 65 cycles and the softmax normalization
    becomes a per-partition reciprocal+scale fused into the PSUM drain.
  - y (token-major) is transposed back per 128x128 block on the PE and
    the projection runs as in the baseline (yT stationary, wp moving).
  - two passes over query halves (i < 1024, i >= 1024) bound es SBUF and
    let first-half projection tiles overlap second-half attention.
  - Engine budget: PE ~154us, ACT ~152us, DVE ~85us, Pool ~50us.
"""

import sys

sys.path.insert(0, "/opt/trn_rl_repo")

from collections import deque
from contextlib import ExitStack

import ml_dtypes
import numpy as np

import concourse.bass as bass
import concourse.mybir as mybir
import concourse.tile as tile
from concourse import bacc
from concourse import bass_utils

f32 = mybir.dt.float32
f32r = mybir.dt.float32r
bf16 = mybir.dt.bfloat16
fp8 = mybir.dt.float8e4
u32 = mybir.dt.uint32
EXP = mybir.ActivationFunctionType.Exp
MUL = mybir.AluOpType.mult
ADD = mybir.AluOpType.add
DR = mybir.MatmulPerfMode.DoubleRow

B, T, C, H, D = 4, 2048, 1024, 16, 64
HL = H // 2          # 8 heads per core
CL = HL * D          # 512 local feature width
P = 128
NJC = T // P         # 16 token chunks of 128

# Weights are pre-scaled by SC on the host so their hi/lo fp8 split stays in
# e4m3's normal range (w ~ N(0, 0.02^2) would otherwise land in subnormals
# where the lo residual quantizes to zero). q/k/v come out SC x too large;
# 1/SC^2 folds into the exp scale and 1/SC into w_proj.
SC = 32.0

# es strip column offsets (packed per head): pass 1 covers i in
# [128jc, 1024), pass 2 covers i in [max(128jc, 1024), 2048).
W1 = [1024 - 128 * jc for jc in range(8)]
O1 = [sum(W1[:jc]) for jc in range(8)]
W2 = [min(1024, 2048 - 128 * jc) for jc in range(16)]
O2 = [sum(W2[:jc]) for jc in range(16)]
ES_COLS = O2[15] + W2[15]  # 12800


def build_body(tc, aps):
    nc = tc.nc

    with ExitStack() as ctx:
        const = ctx.enter_context(tc.tile_pool(name="const", bufs=1))
        xq_pool = ctx.enter_context(tc.tile_pool(name="xq", bufs=1))
        wq_pool = ctx.enter_context(tc.tile_pool(name="wq", bufs=1))
        wv_pool = ctx.enter_context(tc.tile_pool(name="wv", bufs=1))
        qk8_pool = ctx.enter_context(tc.tile_pool(name="qk8", bufs=1))
        vaug_pool = ctx.enter_context(tc.tile_pool(name="vaug", bufs=1))
        es_pool = ctx.enter_context(tc.tile_pool(name="es", bufs=2))
        y2_pool = ctx.enter_context(tc.tile_pool(name="y2", bufs=2))
        yT_pool = ctx.enter_context(tc.tile_pool(name="yT", bufs=1))
        wp_pool = ctx.enter_context(tc.tile_pool(name="wp", bufs=1))
        ostg = ctx.enter_context(tc.tile_pool(name="ostg", bufs=3))
        rc_pool = ctx.enter_context(tc.tile_pool(name="rc", bufs=2))
        psA = ctx.enter_context(tc.tile_pool(name="psA", bufs=2, space="PSUM"))
        psS = ctx.enter_context(tc.tile_pool(name="psS", bufs=2, space="PSUM"))
        psV = ctx.enter_context(tc.tile_pool(name="psV", bufs=2, space="PSUM"))

        # startup DMAs: SP carries bqk + the head-pair-0 wq slices + xh (the
        # critical path to the first strip); Pool carries xl then the rest
        # of wq; ACT stays clean (it is the bottleneck engine).
        bqk_sb = const.tile([P, 8], f32)
        nc.sync.dma_start(bqk_sb[:], aps["bqk"][:])
        wqh_sb = wq_pool.tile([P, 8, 4, 2, P], fp8, name="wqh_sb")
        wql_sb = wq_pool.tile([P, 8, 4, 2, P], fp8, name="wql_sb")
        for jq in (4, 0):
            nc.sync.dma_start(wqh_sb[:, jq], aps["wqh"][:, jq])
            nc.sync.dma_start(wql_sb[:, jq], aps["wql"][:, jq])
        xh_sb = xq_pool.tile([P, 4, 2, T], fp8, name="xh_sb")
        xl_sb = xq_pool.tile([P, 4, 2, T], fp8, name="xl_sb")
        nc.sync.dma_start(xh_sb[:], aps["xh"][:])
        nc.gpsimd.dma_start(xl_sb[:], aps["xl"][:])
        for jq in (1, 5, 2, 6, 3, 7):
            nc.gpsimd.dma_start(wqh_sb[:, jq], aps["wqh"][:, jq])
            nc.gpsimd.dma_start(wql_sb[:, jq], aps["wql"][:, jq])
        masks_sb = const.tile([P, P], bf16)
        nc.sync.dma_start(masks_sb[:], aps["masks"][:])
        wvh_sb = wv_pool.tile([P, 4, 2, CL], fp8, name="wvh_sb")
        wvl_sb = wv_pool.tile([P, 4, 2, CL], fp8, name="wvl_sb")
        nc.sync.dma_start(wvh_sb[:], aps["wvh"][:])
        nc.sync.dma_start(wvl_sb[:], aps["wvl"][:])
        bv_rep = const.tile([P, CL], f32)
        nc.sync.dma_start(bv_rep[:], aps["bv"][None, :].to_broadcast([P, CL]))
        ident_sb = const.tile([P, P], bf16)
        nc.sync.dma_start(ident_sb[:], aps["ident"][:])

        # q/k fp8 pair-plane tiles, one per head-pair u: plane 0 = data,
        # plane 1 = zeros (kills the second DoubleRow term at 64-deep K).
        # Memsets go on DVE: the Pool queue is busy with the xl/wq DMAs.
        q8 = [qk8_pool.tile([P, 2, T], fp8, name=f"q8_{u}") for u in range(4)]
        k8 = [qk8_pool.tile([P, 2, T], fp8, name=f"k8_{u}") for u in range(4)]
        for t in k8 + q8:
            nc.vector.memset(t[:, 1, :].bitcast(u32), 0)

        vaug = vaug_pool.tile([P, NJC, HL, D + 1], bf16)
        nc.vector.memset(vaug[:, :, :, D : D + 1], 1.0)

        yT = yT_pool.tile([P, 4, T], bf16)
        wp_sb = wp_pool.tile([P, 4, C], bf16, name="wp_sb")

        # ---------------- work-unit emitters ----------------
        def qk_chunk(jq, tci):
            # 512 tokens of q (jq<4) or k (jq>=4) chunk -> fp8 store
            ps = psA.tile([P, 512], f32, tag="a")
            first = True
            for kc in range(4):
                for wsb, xsb in ((wqh_sb, xh_sb), (wqh_sb, xl_sb),
                                 (wql_sb, xh_sb)):
                    nc.tensor.matmul(
                        ps[:], wsb[:, jq, kc], xsb[:, kc, :, tci * 512 : tci * 512 + 512],
                        start=first, stop=(kc == 3 and wsb is wql_sb),
                        perf_mode=DR,
                    )
                    first = False
            dest = q8[jq] if jq < 4 else k8[jq - 4]
            nc.vector.tensor_scalar_add(
                dest[:, 0, tci * 512 : tci * 512 + 512], ps[:],
                bqk_sb[:, jq : jq + 1],
            )

        def v_chunk(jc):
            # 128 tokens of v for all 8 heads -> vaug bf16
            ps = psA.tile([P, 512], f32, tag="a")
            first = True
            for kc in range(4):
                for wsb, xsb in ((wvh_sb, xh_sb), (wvh_sb, xl_sb),
                                 (wvl_sb, xh_sb)):
                    nc.tensor.matmul(
                        ps[:], xsb[:, kc, :, jc * P : (jc + 1) * P],
                        wsb[:, kc],
                        start=first, stop=(kc == 3 and wsb is wvl_sb),
                        perf_mode=DR,
                    )
                    first = False
            nc.vector.tensor_tensor(
                vaug[:, jc, :, 0:D],
                ps[:].rearrange("p (h d) -> p h d", h=HL),
                bv_rep[:].rearrange("p (h d) -> p h d", h=HL), ADD,
            )

        def s_strip(h, pas, jc, es_t):
            u, ko = h // 2, 64 * (h % 2)
            i0 = 128 * jc if pas == 1 else max(128 * jc, 1024)
            w = (1024 if pas == 1 else 2048) - i0
            off = O1[jc] if pas == 1 else O2[jc]
            ps = psS.tile([P, 1024], f32, tag="s")
            for c0 in range(0, w, 512):
                n = min(512, w - c0)
                nc.tensor.matmul(
                    ps[:, c0 : c0 + n],
                    k8[u][ko : ko + 64, :, jc * P : (jc + 1) * P],
                    q8[u][ko : ko + 64, :, i0 + c0 : i0 + c0 + n],
                    start=True, stop=True, perf_mode=DR,
                )
            nc.scalar.activation(
                es_t[:, off : off + w], ps[:, 0:w], EXP, scale=0.125 / (SC * SC)
            )
            if pas == 1 or jc >= 8:
                nc.gpsimd.tensor_tensor(
                    es_t[:, off : off + P], es_t[:, off : off + P], masks_sb[:], MUL
                )

        def av_ib(h, pas, ib, es_t, y2t):
            ps = psV.tile([P, 512], f32, tag="av")
            for jc in range(ib + 1):
                if pas == 1:
                    col = O1[jc] + (ib - jc) * P
                else:
                    col = O2[jc] + ib * P - max(128 * jc, 1024)
                nc.tensor.matmul(
                    ps[:, 0 : D + 1],
                    es_t[:, col : col + P],
                    vaug[:, jc, h, :],
                    start=(jc == 0), stop=(jc == ib),
                )
            rc = rc_pool.tile([P, 1], f32, tag="rc")
            nc.vector.reciprocal(rc[:], ps[:, D : D + 1])
            nc.vector.tensor_scalar_mul(
                y2t[:, ib % 8, 64 * (h % 2) : 64 * (h % 2) + 64], ps[:, 0:D], rc[:]
            )

        def transpose_one(u, pas, r, y2t):
            base = 0 if pas == 1 else 8
            pt = psV.tile([P, 512], f32, tag="av")
            ptb = pt[:, 0:64].bitcast(bf16)
            nc.tensor.transpose(ptb, y2t[:, r, :], ident_sb[:])
            nc.vector.tensor_copy(
                yT[:, u, (base + r) * P : (base + r + 1) * P], ptb
            )

        def c_tile(tcb, oc):
            ps = psA.tile([P, 512], f32, tag="a")
            for lc in range(4):
                nc.tensor.matmul(
                    ps[:],
                    yT[:, lc, tcb * P : (tcb + 1) * P],
                    wp_sb[:, lc, oc * 512 : oc * 512 + 512],
                    start=(lc == 0), stop=(lc == 3),
                )
            ot = ostg.tile([P, 512], f32, tag="o")
            nc.vector.tensor_copy(ot[:], ps[:])
            nc.sync.dma_start(
                aps["outp"][tcb * P : (tcb + 1) * P, oc * 512 : oc * 512 + 512],
                ot[:],
            )

        # ---------------- schedule ----------------
        filler = deque()
        state = {"done": 0}

        def need(k):
            while filler and state["done"] < k:
                filler.popleft()()
                state["done"] += 1

        def drip(n=1):
            for _ in range(n):
                if filler:
                    filler.popleft()()
                    state["done"] += 1

        # Only the head-pair-0 chunks pass 1 actually reads (q/k token halves
        # 0 and 1) are emitted directly; their i>=1024 halves join the filler
        # queue. v chunks 8..15 are deferred past the qk chunks: pass-1 AV
        # only reads v[jc<8], and pass-2 has idle PE while ACT churns exp.
        qk_chunk(4, 0)
        qk_chunk(0, 0)
        qk_chunk(0, 1)
        qk_chunk(4, 1)
        filler.extend(                                                     # 0..3
            lambda jq=jq, tci=tci: qk_chunk(jq, tci)
            for jq, tci in ((0, 2), (4, 2), (0, 3), (4, 3))
        )
        filler.extend(lambda jc=jc: v_chunk(jc) for jc in range(8))        # 4..11
        for grp in ((1, 5), (2, 6), (3, 7)):                               # 12..35
            filler.extend(
                lambda jq=jq, tci=tci: qk_chunk(jq, tci)
                for jq in grp for tci in range(4)
            )
        filler.extend(lambda jc=jc: v_chunk(jc) for jc in range(8, NJC))   # 36..43

        # Heads 0..5 are software-pipelined: head h's AV/normalize/transpose
        # work (prev_work) executes interleaved into head h+1's strip loop so
        # the ACT exp stream never waits on a post-strip block. The last PAIR
        # (heads 6,7) interleaves both heads' strips and emits AV (plus
        # transposes/c_tiles in pass 2) as soon as each i-block completes,
        # spreading the would-be tail over the pair's whole exp stream.
        prev_work = deque()
        y2t_box = {}

        def av_need(h, pas, ib):
            if pas == 1:
                need(min(ib, 7) + 5 if h == 0 else 12)
            else:
                need(44)

        for pas in (1, 2):
            njc = 8 if pas == 1 else 16
            for h in range(HL - 2):
                u = h // 2
                es_t = es_pool.tile([P, ES_COLS], bf16, tag="es", name=f"es{pas}_{h}")
                if h % 2 == 0:
                    y2t_box[(pas, u)] = y2_pool.tile(
                        [P, 8, P], bf16, tag="y2", name=f"y2{pas}_{u}"
                    )
                y2t = y2t_box[(pas, u)]
                if pas == 1 and u > 0:
                    need(12 + 8 * u)
                if pas == 2 and h == 0:
                    need(4)
                per = -(-len(prev_work) // njc) if prev_work else 0
                for jc in range(njc):
                    s_strip(h, pas, jc, es_t)
                    for _ in range(per):
                        if prev_work:
                            prev_work.popleft()()
                    drip(1)
                while prev_work:
                    prev_work.popleft()()

                def av_item(ib, h=h, pas=pas, es_t=es_t, y2t=y2t):
                    av_need(h, pas, ib)
                    av_ib(h, pas, ib, es_t, y2t)

                for ib in (range(8) if pas == 1 else range(8, 16)):
                    prev_work.append(lambda ib=ib, f=av_item: f(ib))
                if h % 2 == 1:
                    prev_work.extend(
                        lambda u=u, pas=pas, r=r, y2t=y2t: transpose_one(
                            u, pas, r, y2t
                        )
                        for r in range(8)
                    )
                if pas == 1 and h == 0:
                    nc.sync.dma_start(
                        wp_sb[:], aps["wp"].rearrange("(l p) n -> p l n", p=P)
                    )

            # ---- last pair (heads 6, 7), interleaved ----
            while prev_work:
                prev_work.popleft()()
            es6 = es_pool.tile([P, ES_COLS], bf16, tag="es", name=f"es{pas}_6")
            es7 = es_pool.tile([P, ES_COLS], bf16, tag="es", name=f"es{pas}_7")
            y2t = y2_pool.tile([P, 8, P], bf16, tag="y2", name=f"y2{pas}_3")
            for jc in range(njc):
                s_strip(6, pas, jc, es6)
                drip(1)
                s_strip(7, pas, jc, es7)
                if pas == 1:
                    av_need(6, pas, jc)
                    av_ib(6, pas, jc, es6, y2t)
                    av_ib(7, pas, jc, es7, y2t)
                elif jc >= 8:
                    av_need(6, pas, jc)
                    av_ib(6, pas, jc, es6, y2t)
                    av_ib(7, pas, jc, es7, y2t)
                    transpose_one(3, pas, jc - 8, y2t)
                    c_tile(jc, 0)
                    c_tile(jc, 1)
                else:
                    drip(1)
            if pas == 1:
                for r in range(8):
                    transpose_one(3, pas, r, y2t)
                filler.extend(                                             # 44..59
                    lambda t=t, o=o: c_tile(t, o)
                    for t in range(8) for o in range(2)
                )
        while filler:
            filler.popleft()()


_CACHE = {}


def build_nc():
    if "nc" in _CACHE:
        return _CACHE["nc"]
    nc = bacc.Bacc(
        "TRN2",
        target_bir_lowering=False,
        debug=False,
        enable_asserts=False,
        num_devices=8,
    )
    aps = {
        "xh": nc.dram_tensor("xh", [P, 4, 2, T], fp8, kind="ExternalInput").ap(),
        "xl": nc.dram_tensor("xl", [P, 4, 2, T], fp8, kind="ExternalInput").ap(),
        "wqh": nc.dram_tensor("wqh", [P, 8, 4, 2, P], fp8, kind="ExternalInput").ap(),
        "wql": nc.dram_tensor("wql", [P, 8, 4, 2, P], fp8, kind="ExternalInput").ap(),
        "wvh": nc.dram_tensor("wvh", [P, 4, 2, CL], fp8, kind="ExternalInput").ap(),
        "wvl": nc.dram_tensor("wvl", [P, 4, 2, CL], fp8, kind="ExternalInput").ap(),
        "bqk": nc.dram_tensor("bqk", [P, 8], f32, kind="ExternalInput").ap(),
        "bv": nc.dram_tensor("bv", [CL], f32, kind="ExternalInput").ap(),
        "wp": nc.dram_tensor("wp", [CL, C], bf16, kind="ExternalInput").ap(),
        "masks": nc.dram_tensor("masks", [P, P], bf16, kind="ExternalInput").ap(),
        "ident": nc.dram_tensor("ident", [P, P], bf16, kind="ExternalInput").ap(),
        "outp": nc.dram_tensor("outp", [T, C], f32, kind="ExternalOutput").ap(),
    }
    with tile.TileContext(nc) as tc:
        build_body(tc, aps)
    nc.compile()
    _CACHE["nc"] = nc
    return nc


F8NP = mybir.dt.np(fp8)


def _hi_lo(a):
    hi = a.astype(F8NP)
    lo = (a - hi.astype(np.float32)).astype(F8NP)
    return hi, lo


def _dr_layout(a, free_shape):
    # [C, N...] with contraction c = kc*256 + i*128 + p -> [128, 4, 2, N...]
    return np.ascontiguousarray(
        a.reshape(4, 2, P, *free_shape).transpose(2, 0, 1, 3)
    )


def make_in_maps(x, w_attn, b_attn, w_proj, b_proj):
    masks = np.triu(np.ones((P, P), dtype=np.float32)).astype(ml_dtypes.bfloat16)
    ident = np.eye(P, dtype=np.float32).astype(ml_dtypes.bfloat16)
    in_maps = []
    for core in range(8):
        b, g = core // 2, core % 2
        xT = np.ascontiguousarray(x[b].T)  # [C, T]
        xh, xl = _hi_lo(xT)
        qcols = slice(g * CL, (g + 1) * CL)
        kcols = slice(C + g * CL, C + (g + 1) * CL)
        vcols = slice(2 * C + g * CL, 2 * C + (g + 1) * CL)
        wqk = SC * np.concatenate([w_attn[:, qcols], w_attn[:, kcols]], axis=1)
        wqh, wql = _hi_lo(wqk)
        wvh, wvl = _hi_lo(SC * w_attn[:, vcols])
        bqk = SC * np.concatenate([b_attn[qcols], b_attn[kcols]]).reshape(8, P).T
        in_maps.append(
            {
                "xh": _dr_layout(xh, (T,)),
                "xl": _dr_layout(xl, (T,)),
                # [C, 1024] -> [4, 2, 128p, 8jq, 128m] -> [p, jq, kc, i, m]
                "wqh": np.ascontiguousarray(
                    wqh.reshape(4, 2, P, 8, P).transpose(2, 3, 0, 1, 4)
                ),
                "wql": np.ascontiguousarray(
                    wql.reshape(4, 2, P, 8, P).transpose(2, 3, 0, 1, 4)
                ),
                "wvh": _dr_layout(wvh, (CL,)),
                "wvl": _dr_layout(wvl, (CL,)),
                "bqk": np.ascontiguousarray(bqk),
                "bv": np.ascontiguousarray(SC * b_attn[vcols]),
                "wp": np.ascontiguousarray(
                    (w_proj[g * CL : (g + 1) * CL, :] / SC).astype(ml_dtypes.bfloat16)
                ),
                "masks": masks,
                "ident": ident,
            }
        )
    return in_maps


def combine(parts, b_proj):
    return np.stack(
        [parts[2 * b] + parts[2 * b + 1] + b_proj[None, :] for b in range(B)]
    ).astype(np.float32)


def kernel(x, w_attn, b_attn, w_proj, b_proj, _trace=False, **run_kwargs):
    x = np.asarray(x, dtype=np.float32)
    w_attn = np.asarray(w_attn, dtype=np.float32)
    b_attn = np.asarray(b_attn, dtype=np.float32)
    w_proj = np.asarray(w_proj, dtype=np.float32)
    b_proj = np.asarray(b_proj, dtype=np.float32)

    nc = build_nc()
    in_maps = make_in_maps(x, w_attn, b_attn, w_proj, b_proj)
    try:
        res = bass_utils.run_bass_kernel_spmd(
            nc, in_maps, core_ids=list(range(8)), trace=_trace, **run_kwargs
        )
    except Exception:
        # transient NRT device wedge: one retry
        res = bass_utils.run_bass_kernel_spmd(
            nc, in_maps, core_ids=list(range(8)), trace=_trace, **run_kwargs
        )
    parts = [res.results[i]["outp"] for i in range(8)]
    out = combine(parts, b_proj)
    if _trace:
        return out, res
    return out


# revision 29
# speedup vs baseline: 1.1010x; 1.1010x over previous
"""Causal self-attention (B=4, T=2048, C=1024, H=16, D=64) on 8 trn2 cores.

Sharding: core i handles batch b = i//2 and head-group g = i%2 (8 of 16
heads), tensor-parallel over c_attn columns / c_proj rows. Each core
computes qkv for its heads, causal attention, and a partial projection
(its 512 rows of w_proj); the host sums the two partials per batch and
adds b_proj.

v2 pipeline (per core), built around the engine cost model:
  - q/k/v projections run on the PE in fp8 DoubleRow mode with a hi/lo
    split (x = x_hi + x_lo, w = w_hi + w_lo, three cross terms): 256-wide
    contraction per instruction at 0.5 cyc/col = 2.67x bf16 throughput
    at ~bf16 accuracy.
  - q/k are stored to SBUF as fp8 (e4m3) in a [64d, 2, T] pair-plane
    layout whose second plane is zeroed; S^T strips then also run in
    DoubleRow mode (2 cyc per 4 cols) despite the 64-deep contraction.
  - exp on ACT (the critical engine: ~139k cols x 0.83ns) writes bf16
    es strips; the causal diagonal block is masked by a DVE multiply.
  - AV is token-major: es strip blocks [128j, 128i] are the *stationary*
    operand, v_aug [128j, 65] (ones column -> denominators) the moving
    one, so each block costs 65 cycles and the softmax normalization
    becomes a per-partition reciprocal+scale fused into the PSUM drain.
  - y (token-major) is transposed back per 128x128 block on the PE and
    the projection runs as in the baseline (yT stationary, wp moving).
  - two passes over query halves (i < 1024, i >= 1024) bound es SBUF and
    let first-half projection tiles overlap second-half attention.
  - Engine budget: PE ~154us, ACT ~152us, DVE ~85us, Pool ~50us.
"""

import sys

sys.path.insert(0, "/opt/trn_rl_repo")

from collections import deque
from contextlib import ExitStack

import ml_dtypes
import numpy as np

import concourse.bass as bass
import concourse.mybir as mybir
import concourse.tile as tile
from concourse import bacc
from concourse import bass_utils

f32 = mybir.dt.float32
f32r = mybir.dt.float32r
bf16 = mybir.dt.bfloat16
fp8 = mybir.dt.float8e4
u32 = mybir.dt.uint32
EXP = mybir.ActivationFunctionType.Exp
MUL = mybir.AluOpType.mult
ADD = mybir.AluOpType.add
DR = mybir.MatmulPerfMode.DoubleRow

B, T, C, H, D = 4, 2048, 1024, 16, 64
HL = H // 2          # 8 heads per core
CL = HL * D          # 512 local feature width
P = 128
NJC = T // P         # 16 token chunks of 128

# Weights are pre-scaled by SC on the host so their hi/lo fp8 split stays in
# e4m3's normal range (w ~ N(0, 0.02^2) would otherwise land in subnormals
# where the lo residual quantizes to zero). q/k/v come out SC x too large;
# 1/SC^2 folds into the exp scale and 1/SC into w_proj.
SC = 32.0

# es strip column offsets (packed per head): pass 1 covers i in
# [128jc, 1024), pass 2 covers i in [max(128jc, 1024), 2048).
W1 = [1024 - 128 * jc for jc in range(8)]
O1 = [sum(W1[:jc]) for jc in range(8)]
W2 = [min(1024, 2048 - 128 * jc) for jc in range(16)]
O2 = [sum(W2[:jc]) for jc in range(16)]
ES_COLS = O2[15] + W2[15]  # 12800


def build_body(tc, aps):
    nc = tc.nc

    with ExitStack() as ctx:
        const = ctx.enter_context(tc.tile_pool(name="const", bufs=1))
        xq_pool = ctx.enter_context(tc.tile_pool(name="xq", bufs=1))
        wq_pool = ctx.enter_context(tc.tile_pool(name="wq", bufs=1))
        wv_pool = ctx.enter_context(tc.tile_pool(name="wv", bufs=1))
        qk8_pool = ctx.enter_context(tc.tile_pool(name="qk8", bufs=1))
        vaug_pool = ctx.enter_context(tc.tile_pool(name="vaug", bufs=1))
        es_pool = ctx.enter_context(tc.tile_pool(name="es", bufs=2))
        y2_pool = ctx.enter_context(tc.tile_pool(name="y2", bufs=6))
        yT_pool = ctx.enter_context(tc.tile_pool(name="yT", bufs=1))
        wp_pool = ctx.enter_context(tc.tile_pool(name="wp", bufs=1))
        ostg = ctx.enter_context(tc.tile_pool(name="ostg", bufs=3))
        rc_pool = ctx.enter_context(tc.tile_pool(name="rc", bufs=2))
        psA = ctx.enter_context(tc.tile_pool(name="psA", bufs=2, space="PSUM"))
        psS = ctx.enter_context(tc.tile_pool(name="psS", bufs=2, space="PSUM"))
        psV = ctx.enter_context(tc.tile_pool(name="psV", bufs=2, space="PSUM"))

        # startup DMAs: xh alone leads the SP queue (critical path to the
        # first strip); the tiny bqk + head-pair-0 wq slices ride the ACT
        # queue (idle until ~13us anyway); Pool carries xl then the rest
        # of wq.
        bqk_sb = const.tile([P, 8], f32)
        nc.scalar.dma_start(bqk_sb[:], aps["bqk"][:])
        wqh_sb = wq_pool.tile([P, 8, 4, 2, P], fp8, name="wqh_sb")
        wql_sb = wq_pool.tile([P, 8, 4, 2, P], fp8, name="wql_sb")
        for jq in (4, 0):
            nc.scalar.dma_start(wqh_sb[:, jq], aps["wqh"][:, jq])
            nc.scalar.dma_start(wql_sb[:, jq], aps["wql"][:, jq])
        xh_sb = xq_pool.tile([P, 4, 2, T], fp8, name="xh_sb")
        xl_sb = xq_pool.tile([P, 4, 2, T], fp8, name="xl_sb")
        nc.sync.dma_start(xh_sb[:], aps["xh"][:])
        nc.gpsimd.dma_start(xl_sb[:], aps["xl"][:])
        for jq in (1, 5, 2, 6, 3, 7):
            nc.gpsimd.dma_start(wqh_sb[:, jq], aps["wqh"][:, jq])
            nc.gpsimd.dma_start(wql_sb[:, jq], aps["wql"][:, jq])
        masks_sb = const.tile([P, P], bf16)
        nc.sync.dma_start(masks_sb[:], aps["masks"][:])
        wvh_sb = wv_pool.tile([P, 4, 2, CL], fp8, name="wvh_sb")
        wvl_sb = wv_pool.tile([P, 4, 2, CL], fp8, name="wvl_sb")
        nc.sync.dma_start(wvh_sb[:], aps["wvh"][:])
        nc.sync.dma_start(wvl_sb[:], aps["wvl"][:])
        bv_rep = const.tile([P, CL], f32)
        nc.sync.dma_start(bv_rep[:], aps["bv"][None, :].to_broadcast([P, CL]))
        ident_sb = const.tile([P, P], bf16)
        nc.sync.dma_start(ident_sb[:], aps["ident"][:])

        # q/k fp8 pair-plane tiles, one per head-pair u: plane 0 = data,
        # plane 1 = zeros (kills the second DoubleRow term at 64-deep K).
        # Memsets go on DVE: the Pool queue is busy with the xl/wq DMAs.
        q8 = [qk8_pool.tile([P, 2, T], fp8, name=f"q8_{u}") for u in range(4)]
        k8 = [qk8_pool.tile([P, 2, T], fp8, name=f"k8_{u}") for u in range(4)]
        for t in k8 + q8:
            nc.vector.memset(t[:, 1, :].bitcast(u32), 0)

        vaug = vaug_pool.tile([P, NJC, HL, D + 1], bf16)
        nc.vector.memset(vaug[:, :, :, D : D + 1], 1.0)

        yT = yT_pool.tile([P, 4, T], bf16)
        wp_sb = wp_pool.tile([P, 4, C], bf16, name="wp_sb")

        # ---------------- work-unit emitters ----------------
        def qk_chunk(jq, tci):
            # 512 tokens of q (jq<4) or k (jq>=4) chunk -> fp8 store
            ps = psA.tile([P, 512], f32, tag="a")
            first = True
            for kc in range(4):
                for wsb, xsb in ((wqh_sb, xh_sb), (wqh_sb, xl_sb),
                                 (wql_sb, xh_sb)):
                    nc.tensor.matmul(
                        ps[:], wsb[:, jq, kc], xsb[:, kc, :, tci * 512 : tci * 512 + 512],
                        start=first, stop=(kc == 3 and wsb is wql_sb),
                        perf_mode=DR,
                    )
                    first = False
            dest = q8[jq] if jq < 4 else k8[jq - 4]
            nc.vector.tensor_scalar_add(
                dest[:, 0, tci * 512 : tci * 512 + 512], ps[:],
                bqk_sb[:, jq : jq + 1],
            )

        def v_chunk(jc):
            # 128 tokens of v for all 8 heads -> vaug bf16
            ps = psA.tile([P, 512], f32, tag="a")
            first = True
            for kc in range(4):
                for wsb, xsb in ((wvh_sb, xh_sb), (wvh_sb, xl_sb),
                                 (wvl_sb, xh_sb)):
                    nc.tensor.matmul(
                        ps[:], xsb[:, kc, :, jc * P : (jc + 1) * P],
                        wsb[:, kc],
                        start=first, stop=(kc == 3 and wsb is wvl_sb),
                        perf_mode=DR,
                    )
                    first = False
            nc.vector.tensor_tensor(
                vaug[:, jc, :, 0:D],
                ps[:].rearrange("p (h d) -> p h d", h=HL),
                bv_rep[:].rearrange("p (h d) -> p h d", h=HL), ADD,
            )

        def s_strip(h, pas, jc, es_t):
            u, ko = h // 2, 64 * (h % 2)
            i0 = 128 * jc if pas == 1 else max(128 * jc, 1024)
            w = (1024 if pas == 1 else 2048) - i0
            off = O1[jc] if pas == 1 else O2[jc]
            ps = psS.tile([P, 1024], f32, tag="s")
            for c0 in range(0, w, 512):
                n = min(512, w - c0)
                nc.tensor.matmul(
                    ps[:, c0 : c0 + n],
                    k8[u][ko : ko + 64, :, jc * P : (jc + 1) * P],
                    q8[u][ko : ko + 64, :, i0 + c0 : i0 + c0 + n],
                    start=True, stop=True, perf_mode=DR,
                )
            nc.scalar.activation(
                es_t[:, off : off + w], ps[:, 0:w], EXP, scale=0.125 / (SC * SC)
            )
            if pas == 1 or jc >= 8:
                nc.gpsimd.tensor_tensor(
                    es_t[:, off : off + P], es_t[:, off : off + P], masks_sb[:], MUL
                )

        def av_ib(h, pas, ib, es_t, y2t):
            ps = psV.tile([P, 512], f32, tag="av")
            for jc in range(ib + 1):
                if pas == 1:
                    col = O1[jc] + (ib - jc) * P
                else:
                    col = O2[jc] + ib * P - max(128 * jc, 1024)
                nc.tensor.matmul(
                    ps[:, 0 : D + 1],
                    es_t[:, col : col + P],
                    vaug[:, jc, h, :],
                    start=(jc == 0), stop=(jc == ib),
                )
            rc = rc_pool.tile([P, 1], f32, tag="rc")
            nc.vector.reciprocal(rc[:], ps[:, D : D + 1])
            nc.vector.tensor_scalar_mul(
                y2t[:, ib % 8, 64 * (h % 2) : 64 * (h % 2) + 64], ps[:, 0:D], rc[:]
            )

        def transpose_one(u, pas, r, y2t):
            base = 0 if pas == 1 else 8
            pt = psV.tile([P, 512], f32, tag="av")
            ptb = pt[:, 0:64].bitcast(bf16)
            nc.tensor.transpose(ptb, y2t[:, r, :], ident_sb[:])
            nc.vector.tensor_copy(
                yT[:, u, (base + r) * P : (base + r + 1) * P], ptb
            )

        def c_tile(tcb, oc):
            ps = psA.tile([P, 512], f32, tag="a")
            for lc in range(4):
                nc.tensor.matmul(
                    ps[:],
                    yT[:, lc, tcb * P : (tcb + 1) * P],
                    wp_sb[:, lc, oc * 512 : oc * 512 + 512],
                    start=(lc == 0), stop=(lc == 3),
                )
            ot = ostg.tile([P, 512], f32, tag="o")
            nc.vector.tensor_copy(ot[:], ps[:])
            nc.sync.dma_start(
                aps["outp"][tcb * P : (tcb + 1) * P, oc * 512 : oc * 512 + 512],
                ot[:],
            )

        # ---------------- schedule ----------------
        filler = deque()
        state = {"done": 0}

        def need(k):
            while filler and state["done"] < k:
                filler.popleft()()
                state["done"] += 1

        def drip(n=1):
            for _ in range(n):
                if filler:
                    filler.popleft()()
                    state["done"] += 1

        # Pass 1 only ever reads q/k token-halves 0 and 1 (i < 1024), so only
        # those chunks run during pass 1 (pair 0 directly, pairs 1-3 via
        # fillers); every tc>=2 chunk, v chunks 8..15, the pass-1 transposes
        # and the first-half c_tiles all shift into pass 2, where the PE
        # otherwise idles under ACT's longer exp stream.
        qk_chunk(4, 0)
        qk_chunk(0, 0)
        qk_chunk(0, 1)
        qk_chunk(4, 1)
        filler.extend(lambda jc=jc: v_chunk(jc) for jc in range(8))        # 0..7
        for grp in ((1, 5), (2, 6), (3, 7)):                               # 8..19
            filler.extend(
                lambda jq=jq, tci=tci: qk_chunk(jq, tci)
                for jq in grp for tci in range(2)
            )

        # Heads 0..5 are software-pipelined: head h's AV/normalize/transpose
        # work (prev_work) executes interleaved into head h+1's strip loop so
        # the ACT exp stream never waits on a post-strip block. The last PAIR
        # (heads 6,7) interleaves both heads' strips and emits AV (plus
        # transposes/c_tiles in pass 2) as soon as each i-block completes,
        # spreading the would-be tail over the pair's whole exp stream.
        prev_work = deque()
        y2t_box = {}

        def av_need(h, pas, ib):
            if pas == 1:
                need(min(ib, 7) + 1 if h == 0 else 8)
            else:
                need(44)

        for pas in (1, 2):
            njc = 8 if pas == 1 else 16
            for h in range(HL - 2):
                u = h // 2
                es_t = es_pool.tile([P, ES_COLS], bf16, tag="es", name=f"es{pas}_{h}")
                if h % 2 == 0:
                    y2t_box[(pas, u)] = y2_pool.tile(
                        [P, 8, P], bf16, tag="y2", name=f"y2{pas}_{u}"
                    )
                y2t = y2t_box[(pas, u)]
                if pas == 1 and u > 0:
                    need(8 + 4 * u)
                if pas == 2:
                    need(24 + 4 * u)
                per = -(-len(prev_work) // njc) if prev_work else 0
                for jc in range(njc):
                    s_strip(h, pas, jc, es_t)
                    for _ in range(per):
                        if prev_work:
                            prev_work.popleft()()
                    drip(2 if pas == 2 and h == 0 else 1)
                while prev_work:
                    prev_work.popleft()()

                def av_item(ib, h=h, pas=pas, es_t=es_t, y2t=y2t):
                    av_need(h, pas, ib)
                    av_ib(h, pas, ib, es_t, y2t)

                for ib in (range(8) if pas == 1 else range(8, 16)):
                    prev_work.append(lambda ib=ib, f=av_item: f(ib))
                if h % 2 == 1 and pas == 2:
                    prev_work.extend(
                        lambda u=u, pas=pas, r=r, y2t=y2t: transpose_one(
                            u, pas, r, y2t
                        )
                        for r in range(8)
                    )
                if pas == 1 and h == 0:
                    nc.sync.dma_start(
                        wp_sb[:], aps["wp"].rearrange("(l p) n -> p l n", p=P)
                    )

            if pas == 1:
                # pass-2-era fillers, appended before the pass-1 last pair so
                # its drips prefetch pair-0's tc>=2 qk chunks across the
                # pass boundary. Pass-1 transposes also defer to here.
                filler.extend(                                             # 20..35
                    lambda jq=jq, tci=tci: qk_chunk(jq, tci)
                    for jq in (0, 4, 1, 5, 2, 6, 3, 7) for tci in (2, 3)
                )
                filler.extend(                                             # 36..43
                    lambda jc=jc: v_chunk(jc) for jc in range(8, NJC)
                )
                filler.extend(                                             # 44..67
                    lambda u=u, r=r: transpose_one(u, 1, r, y2t_box[(1, u)])
                    for u in range(3) for r in range(8)
                )

            # ---- last pair (heads 6, 7), interleaved ----
            while prev_work:
                prev_work.popleft()()
            es6 = es_pool.tile([P, ES_COLS], bf16, tag="es", name=f"es{pas}_6")
            es7 = es_pool.tile([P, ES_COLS], bf16, tag="es", name=f"es{pas}_7")
            need(20 if pas == 1 else 36)
            y2t = y2_pool.tile([P, 8, P], bf16, tag="y2", name=f"y2{pas}_3")
            y2t_box[(pas, 3)] = y2t
            for jc in range(njc):
                s_strip(6, pas, jc, es6)
                drip(1)
                s_strip(7, pas, jc, es7)
                if pas == 1:
                    av_need(6, pas, jc)
                    av_ib(6, pas, jc, es6, y2t)
                    av_ib(7, pas, jc, es7, y2t)
                elif jc >= 8:
                    av_need(6, pas, jc)
                    av_ib(6, pas, jc, es6, y2t)
                    av_ib(7, pas, jc, es7, y2t)
                    transpose_one(3, pas, jc - 8, y2t)
                    c_tile(jc, 0)
                    c_tile(jc, 1)
                else:
                    drip(1)
            if pas == 1:
                filler.extend(                                             # 68..75
                    lambda r=r, y2t=y2t: transpose_one(3, 1, r, y2t)
                    for r in range(8)
                )
                filler.extend(                                             # 76..91
                    lambda t=t, o=o: c_tile(t, o)
                    for t in range(8) for o in range(2)
                )
        while filler:
            filler.popleft()()


_CACHE = {}


def build_nc():
    if "nc" in _CACHE:
        return _CACHE["nc"]
    nc = bacc.Bacc(
        "TRN2",
        target_bir_lowering=False,
        debug=False,
        enable_asserts=False,
        num_devices=8,
    )
    aps = {
        "xh": nc.dram_tensor("xh", [P, 4, 2, T], fp8, kind="ExternalInput").ap(),
        "xl": nc.dram_tensor("xl", [P, 4, 2, T], fp8, kind="ExternalInput").ap(),
        "wqh": nc.dram_tensor("wqh", [P, 8, 4, 2, P], fp8, kind="ExternalInput").ap(),
        "wql": nc.dram_tensor("wql", [P, 8, 4, 2, P], fp8, kind="ExternalInput").ap(),
        "wvh": nc.dram_tensor("wvh", [P, 4, 2, CL], fp8, kind="ExternalInput").ap(),
        "wvl": nc.dram_tensor("wvl", [P, 4, 2, CL], fp8, kind="ExternalInput").ap(),
        "bqk": nc.dram_tensor("bqk", [P, 8], f32, kind="ExternalInput").ap(),
        "bv": nc.dram_tensor("bv", [CL], f32, kind="ExternalInput").ap(),
        "wp": nc.dram_tensor("wp", [CL, C], bf16, kind="ExternalInput").ap(),
        "masks": nc.dram_tensor("masks", [P, P], bf16, kind="ExternalInput").ap(),
        "ident": nc.dram_tensor("ident", [P, P], bf16, kind="ExternalInput").ap(),
        "outp": nc.dram_tensor("outp", [T, C], f32, kind="ExternalOutput").ap(),
    }
    with tile.TileContext(nc) as tc:
        build_body(tc, aps)
    nc.compile()
    _CACHE["nc"] = nc
    return nc


F8NP = mybir.dt.np(fp8)


def _hi_lo(a):
    hi = a.astype(F8NP)
    lo = (a - hi.astype(np.float32)).astype(F8NP)
    return hi, lo


def _dr_layout(a, free_shape):
    # [C, N...] with contraction c = kc*256 + i*128 + p -> [128, 4, 2, N...]
    return np.ascontiguousarray(
        a.reshape(4, 2, P, *free_shape).transpose(2, 0, 1, 3)
    )


def make_in_maps(x, w_attn, b_attn, w_proj, b_proj):
    masks = np.triu(np.ones((P, P), dtype=np.float32)).astype(ml_dtypes.bfloat16)
    ident = np.eye(P, dtype=np.float32).astype(ml_dtypes.bfloat16)
    in_maps = []
    for core in range(8):
        b, g = core // 2, core % 2
        xT = np.ascontiguousarray(x[b].T)  # [C, T]
        xh, xl = _hi_lo(xT)
        qcols = slice(g * CL, (g + 1) * CL)
        kcols = slice(C + g * CL, C + (g + 1) * CL)
        vcols = slice(2 * C + g * CL, 2 * C + (g + 1) * CL)
        wqk = SC * np.concatenate([w_attn[:, qcols], w_attn[:, kcols]], axis=1)
        wqh, wql = _hi_lo(wqk)
        wvh, wvl = _hi_lo(SC * w_attn[:, vcols])
        bqk = SC * np.concatenate([b_attn[qcols], b_attn[kcols]]).reshape(8, P).T
        in_maps.append(
            {
                "xh": _dr_layout(xh, (T,)),
                "xl": _dr_layout(xl, (T,)),
                # [C, 1024] -> [4, 2, 128p, 8jq, 128m] -> [p, jq, kc, i, m]
                "wqh": np.ascontiguousarray(
                    wqh.reshape(4, 2, P, 8, P).transpose(2, 3, 0, 1, 4)
                ),
                "wql": np.ascontiguousarray(
                    wql.reshape(4, 2, P, 8, P).transpose(2, 3, 0, 1, 4)
                ),
                "wvh": _dr_layout(wvh, (CL,)),
                "wvl": _dr_layout(wvl, (CL,)),
                "bqk": np.ascontiguousarray(bqk),
                "bv": np.ascontiguousarray(SC * b_attn[vcols]),
                "wp": np.ascontiguousarray(
                    (w_proj[g * CL : (g + 1) * CL, :] / SC).astype(ml_dtypes.bfloat16)
                ),
                "masks": masks,
                "ident": ident,
            }
        )
    return in_maps


def combine(parts, b_proj):
    return np.stack(
        [parts[2 * b] + parts[2 * b + 1] + b_proj[None, :] for b in range(B)]
    ).astype(np.float32)


def kernel(x, w_attn, b_attn, w_proj, b_proj, _trace=False, **run_kwargs):
    x = np.asarray(x, dtype=np.float32)
    w_attn = np.asarray(w_attn, dtype=np.float32)
    b_attn = np.asarray(b_attn, dtype=np.float32)
    w_proj = np.asarray(w_proj, dtype=np.float32)
    b_proj = np.asarray(b_proj, dtype=np.float32)

    nc = build_nc()
    in_maps = make_in_maps(x, w_attn, b_attn, w_proj, b_proj)
    try:
        res = bass_utils.run_bass_kernel_spmd(
            nc, in_maps, core_ids=list(range(8)), trace=_trace, **run_kwargs
        )
    except Exception:
        # transient NRT device wedge: one retry
        res = bass_utils.run_bass_kernel_spmd(
            nc, in_maps, core_ids=list(range(8)), trace=_trace, **run_kwargs
        )
    parts = [res.results[i]["outp"] for i in range(8)]
    out = combine(parts, b_proj)
    if _trace:
        return out, res
    return out


# revision 35
# speedup vs baseline: 1.1519x; 1.0462x over previous
"""Causal self-attention (B=4, T=2048, C=1024, H=16, D=64) on 8 trn2 cores.

Sharding: core i handles batch b = i//2 and head-group g = i%2 (8 of 16
heads), tensor-parallel over c_attn columns / c_proj rows. Each core
computes qkv for its heads, causal attention, and a partial projection
(its 512 rows of w_proj); the host sums the two partials per batch and
adds b_proj.

v2 pipeline (per core), built around the engine cost model:
  - q/k/v projections run on the PE in fp8 DoubleRow mode with a hi/lo
    split (x = x_hi + x_lo, w = w_hi + w_lo, three cross terms): 256-wide
    contraction per instruction at 0.5 cyc/col = 2.67x bf16 throughput
    at ~bf16 accuracy.
  - q/k are stored to SBUF as fp8 (e4m3) in a [64d, 2, T] pair-plane
    layout whose second plane is zeroed; S^T strips then also run in
    DoubleRow mode (2 cyc per 4 cols) despite the 64-deep contraction.
  - exp on ACT (the critical engine: ~139k cols x 0.83ns) writes bf16
    es strips; the causal diagonal block is masked by a DVE multiply.
  - AV is token-major: es strip blocks [128j, 128i] are the *stationary*
    operand, v_aug [128j, 65] (ones column -> denominators) the moving
    one, so each block costs 65 cycles and the softmax normalization
    becomes a per-partition reciprocal+scale fused into the PSUM drain.
  - y (token-major) is transposed back per 128x128 block on the PE and
    the projection runs as in the baseline (yT stationary, wp moving).
  - two passes over query halves (i < 1024, i >= 1024) bound es SBUF and
    let first-half projection tiles overlap second-half attention.
  - Engine budget: PE ~154us, ACT ~152us, DVE ~85us, Pool ~50us.
"""

import sys

sys.path.insert(0, "/opt/trn_rl_repo")

from collections import deque
from contextlib import ExitStack

import ml_dtypes
import numpy as np

import concourse.bass as bass
import concourse.mybir as mybir
import concourse.tile as tile
from concourse import bacc
from concourse import bass_utils

f32 = mybir.dt.float32
f32r = mybir.dt.float32r
bf16 = mybir.dt.bfloat16
fp8 = mybir.dt.float8e4
u32 = mybir.dt.uint32
EXP = mybir.ActivationFunctionType.Exp
MUL = mybir.AluOpType.mult
ADD = mybir.AluOpType.add
DR = mybir.MatmulPerfMode.DoubleRow

B, T, C, H, D = 4, 2048, 1024, 16, 64
HL = H // 2          # 8 heads per core
CL = HL * D          # 512 local feature width
P = 128
NJC = T // P         # 16 token chunks of 128

# Weights are pre-scaled by SC on the host so their hi/lo fp8 split stays in
# e4m3's normal range (w ~ N(0, 0.02^2) would otherwise land in subnormals
# where the lo residual quantizes to zero). q/k/v come out SC x too large;
# 1/SC^2 folds into the exp scale and 1/SC into w_proj.
SC = 32.0

# Strip widths: pass 1 covers i in [128jc, 1024), pass 2 covers
# i in [max(128jc, 1024), 2048).
W1 = [1024 - 128 * jc for jc in range(8)]
W2 = [min(1024, 2048 - 128 * jc) for jc in range(16)]

# Strips are packed into [128, 1024] PSUM tiles in complementary pairs so
# one exp instruction covers both (the second member accumulates onto the
# pending-zeroed second bank with start=False; its first 512+ columns of
# the pair always mark that bank first). GROUPS[pas] = list of jc-tuples.
GROUPS = {
    1: [(0,), (1, 7), (2, 6), (3, 5), (4,)],
    2: [(jc,) for jc in range(8)] + [(8,), (9, 15), (10, 14), (11, 13), (12,)],
}
ES_OFF = {1: {}, 2: {}}
for pas, groups in GROUPS.items():
    w = W1 if pas == 1 else W2
    off = 0
    for grp in groups:
        for jc in grp:
            ES_OFF[pas][jc] = off
            off += w[jc]
ES_COLS = max(ES_OFF[2][jc] + W2[jc] for jc in range(16))  # 12800


def build_body(tc, aps):
    nc = tc.nc

    with ExitStack() as ctx:
        const = ctx.enter_context(tc.tile_pool(name="const", bufs=1))
        xq_pool = ctx.enter_context(tc.tile_pool(name="xq", bufs=1))
        wq_pool = ctx.enter_context(tc.tile_pool(name="wq", bufs=1))
        wv_pool = ctx.enter_context(tc.tile_pool(name="wv", bufs=1))
        qk8_pool = ctx.enter_context(tc.tile_pool(name="qk8", bufs=1))
        vaug_pool = ctx.enter_context(tc.tile_pool(name="vaug", bufs=1))
        es_pool = ctx.enter_context(tc.tile_pool(name="es", bufs=2))
        y2_pool = ctx.enter_context(tc.tile_pool(name="y2", bufs=6))
        yT_pool = ctx.enter_context(tc.tile_pool(name="yT", bufs=1))
        wp_pool = ctx.enter_context(tc.tile_pool(name="wp", bufs=1))
        ostg = ctx.enter_context(tc.tile_pool(name="ostg", bufs=3))
        rc_pool = ctx.enter_context(tc.tile_pool(name="rc", bufs=2))
        psA = ctx.enter_context(tc.tile_pool(name="psA", bufs=2, space="PSUM"))
        psS = ctx.enter_context(tc.tile_pool(name="psS", bufs=2, space="PSUM"))
        psV = ctx.enter_context(tc.tile_pool(name="psV", bufs=2, space="PSUM"))

        # startup DMAs: xh alone leads the SP queue (critical path to the
        # first strip); the tiny bqk + head-pair-0 wq slices ride the ACT
        # queue (idle until ~13us anyway); Pool carries xl then the rest
        # of wq.
        bqk_sb = const.tile([P, 8], f32)
        nc.scalar.dma_start(bqk_sb[:], aps["bqk"][:])
        wqh_sb = wq_pool.tile([P, 8, 4, 2, P], fp8, name="wqh_sb")
        wql_sb = wq_pool.tile([P, 8, 4, 2, P], fp8, name="wql_sb")
        for jq in (4, 0):
            nc.scalar.dma_start(wqh_sb[:, jq], aps["wqh"][:, jq])
            nc.scalar.dma_start(wql_sb[:, jq], aps["wql"][:, jq])
        # kc-split so the first qk chunk's matmuls chase the DMA
        xh_sb = xq_pool.tile([P, 4, 2, T], fp8, name="xh_sb")
        xl_sb = xq_pool.tile([P, 4, 2, T], fp8, name="xl_sb")
        for kc in range(4):
            nc.sync.dma_start(xh_sb[:, kc], aps["xh"][:, kc])
            nc.gpsimd.dma_start(xl_sb[:, kc], aps["xl"][:, kc])
        for jq in (1, 5, 2, 6, 3, 7):
            nc.gpsimd.dma_start(wqh_sb[:, jq], aps["wqh"][:, jq])
            nc.gpsimd.dma_start(wql_sb[:, jq], aps["wql"][:, jq])
        masks_sb = const.tile([P, P], bf16)
        nc.sync.dma_start(masks_sb[:], aps["masks"][:])
        wvh_sb = wv_pool.tile([P, 4, 2, CL], fp8, name="wvh_sb")
        wvl_sb = wv_pool.tile([P, 4, 2, CL], fp8, name="wvl_sb")
        nc.sync.dma_start(wvh_sb[:], aps["wvh"][:])
        nc.sync.dma_start(wvl_sb[:], aps["wvl"][:])
        bv_rep = const.tile([P, CL], f32)
        nc.sync.dma_start(bv_rep[:], aps["bv"][None, :].to_broadcast([P, CL]))
        ident_sb = const.tile([P, P], bf16)
        nc.sync.dma_start(ident_sb[:], aps["ident"][:])

        # q/k fp8 pair-plane tiles, one per head-pair u: plane 0 = data,
        # plane 1 = zeros (kills the second DoubleRow term at 64-deep K).
        # Memsets go on DVE: the Pool queue is busy with the xl/wq DMAs.
        q8 = [qk8_pool.tile([P, 2, T], fp8, name=f"q8_{u}") for u in range(4)]
        k8 = [qk8_pool.tile([P, 2, T], fp8, name=f"k8_{u}") for u in range(4)]
        for t in k8 + q8:
            nc.vector.memset(t[:, 1, :].bitcast(u32), 0)

        vaug = vaug_pool.tile([P, NJC, HL, D + 1], bf16)
        nc.vector.memset(vaug[:, :, :, D : D + 1], 1.0)

        yT = yT_pool.tile([P, 4, T], bf16)
        wp_sb = wp_pool.tile([P, 4, C], bf16, name="wp_sb")

        # ---------------- work-unit emitters ----------------
        def qk_chunk(jq, tci):
            # 512 tokens of q (jq<4) or k (jq>=4) chunk -> fp8 store
            ps = psA.tile([P, 512], f32, tag="a")
            first = True
            for kc in range(4):
                for wsb, xsb in ((wqh_sb, xh_sb), (wqh_sb, xl_sb),
                                 (wql_sb, xh_sb)):
                    nc.tensor.matmul(
                        ps[:], wsb[:, jq, kc], xsb[:, kc, :, tci * 512 : tci * 512 + 512],
                        start=first, stop=(kc == 3 and wsb is wql_sb),
                        perf_mode=DR,
                    )
                    first = False
            dest = q8[jq] if jq < 4 else k8[jq - 4]
            nc.vector.tensor_scalar_add(
                dest[:, 0, tci * 512 : tci * 512 + 512], ps[:],
                bqk_sb[:, jq : jq + 1],
            )

        def v_chunk(jc):
            # 128 tokens of v for all 8 heads -> vaug bf16
            ps = psA.tile([P, 512], f32, tag="a")
            first = True
            for kc in range(4):
                for wsb, xsb in ((wvh_sb, xh_sb), (wvh_sb, xl_sb),
                                 (wvl_sb, xh_sb)):
                    nc.tensor.matmul(
                        ps[:], xsb[:, kc, :, jc * P : (jc + 1) * P],
                        wsb[:, kc],
                        start=first, stop=(kc == 3 and wsb is wvl_sb),
                        perf_mode=DR,
                    )
                    first = False
            nc.vector.tensor_tensor(
                vaug[:, jc, :, 0:D],
                ps[:].rearrange("p (h d) -> p h d", h=HL),
                bv_rep[:].rearrange("p (h d) -> p h d", h=HL), ADD,
            )

        def s_group(h, pas, grp, es_t):
            # one PSUM tile + one exp for a complementary pair of strips
            u, ko = h // 2, 64 * (h % 2)
            poff, wtot, es0 = 0, 0, ES_OFF[pas][grp[0]]
            for mi, jc in enumerate(grp):
                i0 = 128 * jc if pas == 1 else max(128 * jc, 1024)
                w = (1024 if pas == 1 else 2048) - i0
                wtot += w
            ps = psS.tile([P, 1024], f32, tag="s")
            for mi, jc in enumerate(grp):
                i0 = 128 * jc if pas == 1 else max(128 * jc, 1024)
                w = (1024 if pas == 1 else 2048) - i0
                for c0 in range(0, w, 512):
                    n = min(512, w - c0)
                    nc.tensor.matmul(
                        ps[:, poff + c0 : poff + c0 + n],
                        k8[u][ko : ko + 64, :, jc * P : (jc + 1) * P],
                        q8[u][ko : ko + 64, :, i0 + c0 : i0 + c0 + n],
                        start=(mi == 0), stop=True, perf_mode=DR,
                        skip_group_check=(mi > 0),
                    )
                poff += w
            nc.scalar.activation(
                es_t[:, es0 : es0 + wtot], ps[:, 0:wtot], EXP,
                scale=0.125 / (SC * SC),
            )
            for jc in grp:
                if pas == 1 or jc >= 8:
                    off = ES_OFF[pas][jc]
                    nc.gpsimd.tensor_tensor(
                        es_t[:, off : off + P], es_t[:, off : off + P],
                        masks_sb[:], MUL,
                    )

        def av_ib(h, pas, ib, es_t, y2t):
            ps = psV.tile([P, 512], f32, tag="av")
            for jc in range(ib + 1):
                if pas == 1:
                    col = ES_OFF[1][jc] + (ib - jc) * P
                else:
                    col = ES_OFF[2][jc] + ib * P - max(128 * jc, 1024)
                nc.tensor.matmul(
                    ps[:, 0 : D + 1],
                    es_t[:, col : col + P],
                    vaug[:, jc, h, :],
                    start=(jc == 0), stop=(jc == ib),
                )
            rc = rc_pool.tile([P, 1], f32, tag="rc")
            nc.vector.reciprocal(rc[:], ps[:, D : D + 1])
            nc.vector.tensor_scalar_mul(
                y2t[:, ib % 8, 64 * (h % 2) : 64 * (h % 2) + 64], ps[:, 0:D], rc[:]
            )

        def transpose_one(u, pas, r, y2t):
            base = 0 if pas == 1 else 8
            pt = psV.tile([P, 512], f32, tag="av")
            ptb = pt[:, 0:64].bitcast(bf16)
            nc.tensor.transpose(ptb, y2t[:, r, :], ident_sb[:])
            nc.vector.tensor_copy(
                yT[:, u, (base + r) * P : (base + r + 1) * P], ptb
            )

        def c_tile(tcb, oc):
            ps = psA.tile([P, 512], f32, tag="a")
            for lc in range(4):
                nc.tensor.matmul(
                    ps[:],
                    yT[:, lc, tcb * P : (tcb + 1) * P],
                    wp_sb[:, lc, oc * 512 : oc * 512 + 512],
                    start=(lc == 0), stop=(lc == 3),
                )
            ot = ostg.tile([P, 512], f32, tag="o")
            nc.vector.tensor_copy(ot[:], ps[:])
            nc.sync.dma_start(
                aps["outp"][tcb * P : (tcb + 1) * P, oc * 512 : oc * 512 + 512],
                ot[:],
            )

        # ---------------- schedule ----------------
        filler = deque()
        state = {"done": 0}

        def need(k):
            while filler and state["done"] < k:
                filler.popleft()()
                state["done"] += 1

        def drip(n=1):
            for _ in range(n):
                if filler:
                    filler.popleft()()
                    state["done"] += 1

        # Pass 1 only ever reads q/k token-halves 0 and 1 (i < 1024), so only
        # those chunks run during pass 1 (pair 0 directly, pairs 1-3 via
        # fillers); every tc>=2 chunk, v chunks 8..15, the pass-1 transposes
        # and the first-half c_tiles all shift into pass 2, where the PE
        # otherwise idles under ACT's longer exp stream.
        qk_chunk(4, 0)
        qk_chunk(0, 0)
        qk_chunk(0, 1)
        qk_chunk(4, 1)
        filler.extend(lambda jc=jc: v_chunk(jc) for jc in range(8))        # 0..7
        for grp in ((1, 5), (2, 6), (3, 7)):                               # 8..19
            filler.extend(
                lambda jq=jq, tci=tci: qk_chunk(jq, tci)
                for jq in grp for tci in range(2)
            )

        # Heads 0..5 are software-pipelined: head h's AV/normalize/transpose
        # work (prev_work) executes interleaved into head h+1's strip loop so
        # the ACT exp stream never waits on a post-strip block. The last PAIR
        # (heads 6,7) interleaves both heads' strips and emits AV (plus
        # transposes/c_tiles in pass 2) as soon as each i-block completes,
        # spreading the would-be tail over the pair's whole exp stream.
        prev_work = deque()
        y2t_box = {}

        def av_need(h, pas, ib):
            if pas == 1:
                need(min(ib, 7) + 1 if h == 0 else 8)
            else:
                need(44)

        for pas in (1, 2):
            njc = 8 if pas == 1 else 16
            for h in range(HL - 2):
                u = h // 2
                es_t = es_pool.tile([P, ES_COLS], bf16, tag="es", name=f"es{pas}_{h}")
                if h % 2 == 0:
                    y2t_box[(pas, u)] = y2_pool.tile(
                        [P, 8, P], bf16, tag="y2", name=f"y2{pas}_{u}"
                    )
                y2t = y2t_box[(pas, u)]
                if pas == 1 and u > 0:
                    need(8 + 4 * u)
                if pas == 2:
                    need(24 + 4 * u)
                groups = GROUPS[pas]
                per = -(-len(prev_work) // len(groups)) if prev_work else 0
                for grp in groups:
                    s_group(h, pas, grp, es_t)
                    for _ in range(per):
                        if prev_work:
                            prev_work.popleft()()
                    drip(2 if pas == 2 and h == 0 else 1)
                while prev_work:
                    prev_work.popleft()()

                def av_item(ib, h=h, pas=pas, es_t=es_t, y2t=y2t):
                    av_need(h, pas, ib)
                    av_ib(h, pas, ib, es_t, y2t)

                for ib in (range(8) if pas == 1 else range(8, 16)):
                    prev_work.append(lambda ib=ib, f=av_item: f(ib))
                if h % 2 == 1 and pas == 2:
                    prev_work.extend(
                        lambda u=u, pas=pas, r=r, y2t=y2t: transpose_one(
                            u, pas, r, y2t
                        )
                        for r in range(8)
                    )
                if pas == 1 and h == 0:
                    nc.sync.dma_start(
                        wp_sb[:], aps["wp"].rearrange("(l p) n -> p l n", p=P)
                    )

            if pas == 1:
                # pass-2-era fillers, appended before the pass-1 last pair so
                # its drips prefetch pair-0's tc>=2 qk chunks across the
                # pass boundary. Pass-1 transposes also defer to here.
                filler.extend(                                             # 20..35
                    lambda jq=jq, tci=tci: qk_chunk(jq, tci)
                    for jq in (0, 4, 1, 5, 2, 6, 3, 7) for tci in (2, 3)
                )
                filler.extend(                                             # 36..43
                    lambda jc=jc: v_chunk(jc) for jc in range(8, NJC)
                )
                filler.extend(                                             # 44..67
                    lambda u=u, r=r: transpose_one(u, 1, r, y2t_box[(1, u)])
                    for u in range(3) for r in range(8)
                )

            # ---- last pair (heads 6, 7), interleaved ----
            while prev_work:
                prev_work.popleft()()
            es6 = es_pool.tile([P, ES_COLS], bf16, tag="es", name=f"es{pas}_6")
            es7 = es_pool.tile([P, ES_COLS], bf16, tag="es", name=f"es{pas}_7")
            need(20 if pas == 1 else 36)
            y2t = y2_pool.tile([P, 8, P], bf16, tag="y2", name=f"y2{pas}_3")
            y2t_box[(pas, 3)] = y2t
            # pass 2 keeps single strips here: merged groups would bunch the
            # inline c_tiles at the very end of the kernel
            pair_groups = GROUPS[1] if pas == 1 else [(jc,) for jc in range(16)]
            done_jc, next_ib = set(), 0 if pas == 1 else 8
            for grp in pair_groups:
                s_group(6, pas, grp, es6)
                drip(1)
                s_group(7, pas, grp, es7)
                done_jc.update(grp)
                emitted = False
                while next_ib < njc and all(
                    jc in done_jc for jc in range(next_ib + 1)
                ):
                    ib, next_ib = next_ib, next_ib + 1
                    emitted = True
                    av_need(6, pas, ib)
                    av_ib(6, pas, ib, es6, y2t)
                    av_ib(7, pas, ib, es7, y2t)
                    if pas == 2:
                        transpose_one(3, pas, ib - 8, y2t)
                        c_tile(ib, 0)
                        c_tile(ib, 1)
                if not emitted:
                    drip(1)
            if pas == 1:
                filler.extend(                                             # 68..75
                    lambda r=r, y2t=y2t: transpose_one(3, 1, r, y2t)
                    for r in range(8)
                )
                filler.extend(                                             # 76..91
                    lambda t=t, o=o: c_tile(t, o)
                    for t in range(8) for o in range(2)
                )
        while filler:
            filler.popleft()()


_CACHE = {}


def build_nc():
    if "nc" in _CACHE:
        return _CACHE["nc"]
    nc = bacc.Bacc(
        "TRN2",
        target_bir_lowering=False,
        debug=False,
        enable_asserts=False,
        num_devices=8,
    )
    aps = {
        "xh": nc.dram_tensor("xh", [P, 4, 2, T], fp8, kind="ExternalInput").ap(),
        "xl": nc.dram_tensor("xl", [P, 4, 2, T], fp8, kind="ExternalInput").ap(),
        "wqh": nc.dram_tensor("wqh", [P, 8, 4, 2, P], fp8, kind="ExternalInput").ap(),
        "wql": nc.dram_tensor("wql", [P, 8, 4, 2, P], fp8, kind="ExternalInput").ap(),
        "wvh": nc.dram_tensor("wvh", [P, 4, 2, CL], fp8, kind="ExternalInput").ap(),
        "wvl": nc.dram_tensor("wvl", [P, 4, 2, CL], fp8, kind="ExternalInput").ap(),
        "bqk": nc.dram_tensor("bqk", [P, 8], f32, kind="ExternalInput").ap(),
        "bv": nc.dram_tensor("bv", [CL], f32, kind="ExternalInput").ap(),
        "wp": nc.dram_tensor("wp", [CL, C], bf16, kind="ExternalInput").ap(),
        "masks": nc.dram_tensor("masks", [P, P], bf16, kind="ExternalInput").ap(),
        "ident": nc.dram_tensor("ident", [P, P], bf16, kind="ExternalInput").ap(),
        "outp": nc.dram_tensor("outp", [T, C], f32, kind="ExternalOutput").ap(),
    }
    with tile.TileContext(nc) as tc:
        build_body(tc, aps)
    nc.compile()
    _CACHE["nc"] = nc
    return nc


F8NP = mybir.dt.np(fp8)


def _hi_lo(a):
    hi = a.astype(F8NP)
    lo = (a - hi.astype(np.float32)).astype(F8NP)
    return hi, lo


def _dr_layout(a, free_shape):
    # [C, N...] with contraction c = kc*256 + i*128 + p -> [128, 4, 2, N...]
    return np.ascontiguousarray(
        a.reshape(4, 2, P, *free_shape).transpose(2, 0, 1, 3)
    )


def make_in_maps(x, w_attn, b_attn, w_proj, b_proj):
    masks = np.triu(np.ones((P, P), dtype=np.float32)).astype(ml_dtypes.bfloat16)
    ident = np.eye(P, dtype=np.float32).astype(ml_dtypes.bfloat16)
    in_maps = []
    for core in range(8):
        b, g = core // 2, core % 2
        xT = np.ascontiguousarray(x[b].T)  # [C, T]
        xh, xl = _hi_lo(xT)
        qcols = slice(g * CL, (g + 1) * CL)
        kcols = slice(C + g * CL, C + (g + 1) * CL)
        vcols = slice(2 * C + g * CL, 2 * C + (g + 1) * CL)
        wqk = SC * np.concatenate([w_attn[:, qcols], w_attn[:, kcols]], axis=1)
        wqh, wql = _hi_lo(wqk)
        wvh, wvl = _hi_lo(SC * w_attn[:, vcols])
        bqk = SC * np.concatenate([b_attn[qcols], b_attn[kcols]]).reshape(8, P).T
        in_maps.append(
            {
                "xh": _dr_layout(xh, (T,)),
                "xl": _dr_layout(xl, (T,)),
                # [C, 1024] -> [4, 2, 128p, 8jq, 128m] -> [p, jq, kc, i, m]
                "wqh": np.ascontiguousarray(
                    wqh.reshape(4, 2, P, 8, P).transpose(2, 3, 0, 1, 4)
                ),
                "wql": np.ascontiguousarray(
                    wql.reshape(4, 2, P, 8, P).transpose(2, 3, 0, 1, 4)
                ),
                "wvh": _dr_layout(wvh, (CL,)),
                "wvl": _dr_layout(wvl, (CL,)),
                "bqk": np.ascontiguousarray(bqk),
                "bv": np.ascontiguousarray(SC * b_attn[vcols]),
                "wp": np.ascontiguousarray(
                    (w_proj[g * CL : (g + 1) * CL, :] / SC).astype(ml_dtypes.bfloat16)
                ),
                "masks": masks,
                "ident": ident,
            }
        )
    return in_maps


def combine(parts, b_proj):
    return np.stack(
        [parts[2 * b] + parts[2 * b + 1] + b_proj[None, :] for b in range(B)]
    ).astype(np.float32)


def kernel(x, w_attn, b_attn, w_proj, b_proj, _trace=False, **run_kwargs):
    x = np.asarray(x, dtype=np.float32)
    w_attn = np.asarray(w_attn, dtype=np.float32)
    b_attn = np.asarray(b_attn, dtype=np.float32)
    w_proj = np.asarray(w_proj, dtype=np.float32)
    b_proj = np.asarray(b_proj, dtype=np.float32)

    nc = build_nc()
    in_maps = make_in_maps(x, w_attn, b_attn, w_proj, b_proj)
    try:
        res = bass_utils.run_bass_kernel_spmd(
            nc, in_maps, core_ids=list(range(8)), trace=_trace, **run_kwargs
        )
    except Exception:
        # transient NRT device wedge: one retry
        res = bass_utils.run_bass_kernel_spmd(
            nc, in_maps, core_ids=list(range(8)), trace=_trace, **run_kwargs
        )
    parts = [res.results[i]["outp"] for i in range(8)]
    out = combine(parts, b_proj)
    if _trace:
        return out, res
    return out


# revision 40
# speedup vs baseline: 1.1546x; 1.0024x over previous
"""Causal self-attention (B=4, T=2048, C=1024, H=16, D=64) on 8 trn2 cores.

Sharding: core i handles batch b = i//2 and head-group g = i%2 (8 of 16
heads), tensor-parallel over c_attn columns / c_proj rows. Each core
computes qkv for its heads, causal attention, and a partial projection
(its 512 rows of w_proj); the host sums the two partials per batch and
adds b_proj.

v2 pipeline (per core), built around the engine cost model:
  - q/k/v projections run on the PE in fp8 DoubleRow mode with a hi/lo
    split (x = x_hi + x_lo, w = w_hi + w_lo, three cross terms): 256-wide
    contraction per instruction at 0.5 cyc/col = 2.67x bf16 throughput
    at ~bf16 accuracy.
  - q/k are stored to SBUF as fp8 (e4m3) in a [64d, 2, T] pair-plane
    layout whose second plane is zeroed; S^T strips then also run in
    DoubleRow mode (2 cyc per 4 cols) despite the 64-deep contraction.
  - exp on ACT (the critical engine: ~139k cols x 0.83ns) writes bf16
    es strips; the causal diagonal block is masked by a DVE multiply.
  - AV is token-major: es strip blocks [128j, 128i] are the *stationary*
    operand, v_aug [128j, 65] (ones column -> denominators) the moving
    one, so each block costs 65 cycles and the softmax normalization
    becomes a per-partition reciprocal+scale fused into the PSUM drain.
  - y (token-major) is transposed back per 128x128 block on the PE and
    the projection runs as in the baseline (yT stationary, wp moving).
  - two passes over query halves (i < 1024, i >= 1024) bound es SBUF and
    let first-half projection tiles overlap second-half attention.
  - Engine budget: PE ~154us, ACT ~152us, DVE ~85us, Pool ~50us.
"""

import sys

sys.path.insert(0, "/opt/trn_rl_repo")

from collections import deque
from contextlib import ExitStack

import ml_dtypes
import numpy as np

import concourse.bass as bass
import concourse.mybir as mybir
import concourse.tile as tile
from concourse import bacc
from concourse import bass_utils

f32 = mybir.dt.float32
f32r = mybir.dt.float32r
bf16 = mybir.dt.bfloat16
fp8 = mybir.dt.float8e4
u32 = mybir.dt.uint32
EXP = mybir.ActivationFunctionType.Exp
MUL = mybir.AluOpType.mult
ADD = mybir.AluOpType.add
DR = mybir.MatmulPerfMode.DoubleRow

B, T, C, H, D = 4, 2048, 1024, 16, 64
HL = H // 2          # 8 heads per core
CL = HL * D          # 512 local feature width
P = 128
NJC = T // P         # 16 token chunks of 128

# Weights are pre-scaled by SC on the host so their hi/lo fp8 split stays in
# e4m3's normal range (w ~ N(0, 0.02^2) would otherwise land in subnormals
# where the lo residual quantizes to zero). q/k/v come out SC x too large;
# 1/SC^2 folds into the exp scale and 1/SC into w_proj.
SC = 32.0

# Strip widths: pass 1 covers i in [128jc, 1024), pass 2 covers
# i in [max(128jc, 1024), 2048).
W1 = [1024 - 128 * jc for jc in range(8)]
W2 = [min(1024, 2048 - 128 * jc) for jc in range(16)]

# Strips are packed into [128, 1024] PSUM tiles in complementary pairs so
# one exp instruction covers both (the second member accumulates onto the
# pending-zeroed second bank with start=False; its first 512+ columns of
# the pair always mark that bank first). GROUPS[pas] = list of jc-tuples.
GROUPS = {
    1: [(0,), (1, 7), (2, 6), (3, 5), (4,)],
    2: [(jc,) for jc in range(8)] + [(8,), (9, 15), (10, 14), (11, 13), (12,)],
}
ES_OFF = {1: {}, 2: {}}
for pas, groups in GROUPS.items():
    w = W1 if pas == 1 else W2
    off = 0
    for grp in groups:
        for jc in grp:
            ES_OFF[pas][jc] = off
            off += w[jc]
ES_COLS = max(ES_OFF[2][jc] + W2[jc] for jc in range(16))  # 12800


def build_body(tc, aps):
    nc = tc.nc

    with ExitStack() as ctx:
        const = ctx.enter_context(tc.tile_pool(name="const", bufs=1))
        xq_pool = ctx.enter_context(tc.tile_pool(name="xq", bufs=1))
        wq_pool = ctx.enter_context(tc.tile_pool(name="wq", bufs=1))
        wv_pool = ctx.enter_context(tc.tile_pool(name="wv", bufs=1))
        qk8_pool = ctx.enter_context(tc.tile_pool(name="qk8", bufs=1))
        vaug_pool = ctx.enter_context(tc.tile_pool(name="vaug", bufs=1))
        es_pool = ctx.enter_context(tc.tile_pool(name="es", bufs=2))
        y2_pool = ctx.enter_context(tc.tile_pool(name="y2", bufs=6))
        yT_pool = ctx.enter_context(tc.tile_pool(name="yT", bufs=1))
        wp_pool = ctx.enter_context(tc.tile_pool(name="wp", bufs=1))
        ostg = ctx.enter_context(tc.tile_pool(name="ostg", bufs=3))
        rc_pool = ctx.enter_context(tc.tile_pool(name="rc", bufs=2))
        psA = ctx.enter_context(tc.tile_pool(name="psA", bufs=2, space="PSUM"))
        psS = ctx.enter_context(tc.tile_pool(name="psS", bufs=2, space="PSUM"))
        psV = ctx.enter_context(tc.tile_pool(name="psV", bufs=2, space="PSUM"))

        # startup DMAs: xh alone leads the SP queue (critical path to the
        # first strip); the tiny bqk + head-pair-0 wq slices ride the ACT
        # queue (idle until ~13us anyway); Pool carries xl then the rest
        # of wq.
        bqk_sb = const.tile([P, 8], f32)
        nc.scalar.dma_start(bqk_sb[:], aps["bqk"][:])
        wqh_sb = wq_pool.tile([P, 8, 4, 2, P], fp8, name="wqh_sb")
        wql_sb = wq_pool.tile([P, 8, 4, 2, P], fp8, name="wql_sb")
        for jq in (4, 0):
            nc.scalar.dma_start(wqh_sb[:, jq], aps["wqh"][:, jq])
            nc.scalar.dma_start(wql_sb[:, jq], aps["wql"][:, jq])
        # kc-split so the first qk chunk's matmuls chase the DMA
        xh_sb = xq_pool.tile([P, 4, 2, T], fp8, name="xh_sb")
        xl_sb = xq_pool.tile([P, 4, 2, T], fp8, name="xl_sb")
        for kc in range(4):
            nc.sync.dma_start(xh_sb[:, kc], aps["xh"][:, kc])
            nc.gpsimd.dma_start(xl_sb[:, kc], aps["xl"][:, kc])
        for jq in (1, 5, 2, 6, 3, 7):
            nc.gpsimd.dma_start(wqh_sb[:, jq], aps["wqh"][:, jq])
            nc.gpsimd.dma_start(wql_sb[:, jq], aps["wql"][:, jq])
        masks_sb = const.tile([P, P], bf16)
        nc.sync.dma_start(masks_sb[:], aps["masks"][:])
        wvh_sb = wv_pool.tile([P, 4, 2, CL], fp8, name="wvh_sb")
        wvl_sb = wv_pool.tile([P, 4, 2, CL], fp8, name="wvl_sb")
        nc.sync.dma_start(wvh_sb[:], aps["wvh"][:])
        nc.sync.dma_start(wvl_sb[:], aps["wvl"][:])
        bv_rep = const.tile([P, CL], f32)
        nc.sync.dma_start(bv_rep[:], aps["bv"][None, :].to_broadcast([P, CL]))

        # q/k fp8 pair-plane tiles, one per head-pair u: plane 0 = data,
        # plane 1 = zeros (kills the second DoubleRow term at 64-deep K).
        # Memsets go on DVE: the Pool queue is busy with the xl/wq DMAs.
        q8 = [qk8_pool.tile([P, 2, T], fp8, name=f"q8_{u}") for u in range(4)]
        k8 = [qk8_pool.tile([P, 2, T], fp8, name=f"k8_{u}") for u in range(4)]
        for t in k8 + q8:
            nc.vector.memset(t[:, 1, :].bitcast(u32), 0)

        vaug = vaug_pool.tile([P, NJC, HL, D + 1], bf16)
        nc.vector.memset(vaug[:, :, :, D : D + 1], 1.0)

        yT = yT_pool.tile([P, 4, T], bf16)
        wp_sb = wp_pool.tile([P, 4, C], bf16, name="wp_sb")

        # ---------------- work-unit emitters ----------------
        def qk_chunk(jq, tci):
            # 512 tokens of q (jq<4) or k (jq>=4) chunk -> fp8 store
            ps = psA.tile([P, 512], f32, tag="a")
            first = True
            for kc in range(4):
                for wsb, xsb in ((wqh_sb, xh_sb), (wqh_sb, xl_sb),
                                 (wql_sb, xh_sb)):
                    nc.tensor.matmul(
                        ps[:], wsb[:, jq, kc], xsb[:, kc, :, tci * 512 : tci * 512 + 512],
                        start=first, stop=(kc == 3 and wsb is wql_sb),
                        perf_mode=DR,
                    )
                    first = False
            dest = q8[jq] if jq < 4 else k8[jq - 4]
            nc.vector.tensor_scalar_add(
                dest[:, 0, tci * 512 : tci * 512 + 512], ps[:],
                bqk_sb[:, jq : jq + 1],
            )

        def v_chunk(jc):
            # 128 tokens of v for all 8 heads -> vaug bf16
            ps = psA.tile([P, 512], f32, tag="a")
            first = True
            for kc in range(4):
                for wsb, xsb in ((wvh_sb, xh_sb), (wvh_sb, xl_sb),
                                 (wvl_sb, xh_sb)):
                    nc.tensor.matmul(
                        ps[:], xsb[:, kc, :, jc * P : (jc + 1) * P],
                        wsb[:, kc],
                        start=first, stop=(kc == 3 and wsb is wvl_sb),
                        perf_mode=DR,
                    )
                    first = False
            nc.vector.tensor_tensor(
                vaug[:, jc, :, 0:D],
                ps[:].rearrange("p (h d) -> p h d", h=HL),
                bv_rep[:].rearrange("p (h d) -> p h d", h=HL), ADD,
            )

        def s_group(h, pas, grp, es_t):
            # one PSUM tile + one exp for a complementary pair of strips
            u, ko = h // 2, 64 * (h % 2)
            poff, wtot, es0 = 0, 0, ES_OFF[pas][grp[0]]
            for mi, jc in enumerate(grp):
                i0 = 128 * jc if pas == 1 else max(128 * jc, 1024)
                w = (1024 if pas == 1 else 2048) - i0
                wtot += w
            ps = psS.tile([P, 1024], f32, tag="s")
            for mi, jc in enumerate(grp):
                i0 = 128 * jc if pas == 1 else max(128 * jc, 1024)
                w = (1024 if pas == 1 else 2048) - i0
                for c0 in range(0, w, 512):
                    n = min(512, w - c0)
                    nc.tensor.matmul(
                        ps[:, poff + c0 : poff + c0 + n],
                        k8[u][ko : ko + 64, :, jc * P : (jc + 1) * P],
                        q8[u][ko : ko + 64, :, i0 + c0 : i0 + c0 + n],
                        start=(mi == 0), stop=True, perf_mode=DR,
                        skip_group_check=(mi > 0),
                    )
                poff += w
            nc.scalar.activation(
                es_t[:, es0 : es0 + wtot], ps[:, 0:wtot], EXP,
                scale=0.125 / (SC * SC),
            )
            for jc in grp:
                if pas == 1 or jc >= 8:
                    off = ES_OFF[pas][jc]
                    nc.gpsimd.tensor_tensor(
                        es_t[:, off : off + P], es_t[:, off : off + P],
                        masks_sb[:], MUL,
                    )

        def av_ib(h, pas, ib, es_t, y2t):
            ps = psV.tile([P, 512], f32, tag="av")
            for jc in range(ib + 1):
                if pas == 1:
                    col = ES_OFF[1][jc] + (ib - jc) * P
                else:
                    col = ES_OFF[2][jc] + ib * P - max(128 * jc, 1024)
                nc.tensor.matmul(
                    ps[:, 0 : D + 1],
                    es_t[:, col : col + P],
                    vaug[:, jc, h, :],
                    start=(jc == 0), stop=(jc == ib),
                )
            rc = rc_pool.tile([P, 1], f32, tag="rc")
            nc.vector.reciprocal(rc[:], ps[:, D : D + 1])
            nc.vector.tensor_scalar_mul(
                y2t[:, ib % 8, 64 * (h % 2) : 64 * (h % 2) + 64], ps[:, 0:D], rc[:]
            )

        def transpose_one(u, pas, r, y2t):
            # XBAR DMA transpose: off the PE/DVE critical engines entirely
            base = 0 if pas == 1 else 8
            nc.sync.dma_start_transpose(
                yT[:, u, (base + r) * P : (base + r + 1) * P], y2t[:, r, :]
            )

        def c_tile(tcb, oc):
            ps = psA.tile([P, 512], f32, tag="a")
            for lc in range(4):
                nc.tensor.matmul(
                    ps[:],
                    yT[:, lc, tcb * P : (tcb + 1) * P],
                    wp_sb[:, lc, oc * 512 : oc * 512 + 512],
                    start=(lc == 0), stop=(lc == 3),
                )
            ot = ostg.tile([P, 512], f32, tag="o")
            nc.vector.tensor_copy(ot[:], ps[:])
            nc.sync.dma_start(
                aps["outp"][tcb * P : (tcb + 1) * P, oc * 512 : oc * 512 + 512],
                ot[:],
            )

        # ---------------- schedule ----------------
        filler = deque()
        state = {"done": 0}

        def need(k):
            while filler and state["done"] < k:
                filler.popleft()()
                state["done"] += 1

        def drip(n=1):
            for _ in range(n):
                if filler:
                    filler.popleft()()
                    state["done"] += 1

        # Pass 1 only ever reads q/k token-halves 0 and 1 (i < 1024), so only
        # those chunks run during pass 1 (pair 0 directly, pairs 1-3 via
        # fillers); every tc>=2 chunk, v chunks 8..15, the pass-1 transposes
        # and the first-half c_tiles all shift into pass 2, where the PE
        # otherwise idles under ACT's longer exp stream.
        qk_chunk(4, 0)
        qk_chunk(0, 0)
        qk_chunk(0, 1)
        qk_chunk(4, 1)
        filler.extend(lambda jc=jc: v_chunk(jc) for jc in range(8))        # 0..7
        for grp in ((1, 5), (2, 6), (3, 7)):                               # 8..19
            filler.extend(
                lambda jq=jq, tci=tci: qk_chunk(jq, tci)
                for jq in grp for tci in range(2)
            )

        # Heads 0..5 are software-pipelined: head h's AV/normalize/transpose
        # work (prev_work) executes interleaved into head h+1's strip loop so
        # the ACT exp stream never waits on a post-strip block. The last PAIR
        # (heads 6,7) interleaves both heads' strips and emits AV (plus
        # transposes/c_tiles in pass 2) as soon as each i-block completes,
        # spreading the would-be tail over the pair's whole exp stream.
        prev_work = deque()
        y2t_box = {}

        def av_need(h, pas, ib):
            if pas == 1:
                need(min(ib, 7) + 1 if h == 0 else 8)
            else:
                need(44)

        for pas in (1, 2):
            njc = 8 if pas == 1 else 16
            for h in range(HL - 2):
                u = h // 2
                es_t = es_pool.tile([P, ES_COLS], bf16, tag="es", name=f"es{pas}_{h}")
                if h % 2 == 0:
                    y2t_box[(pas, u)] = y2_pool.tile(
                        [P, 8, P], bf16, tag="y2", name=f"y2{pas}_{u}"
                    )
                y2t = y2t_box[(pas, u)]
                if pas == 1 and u > 0:
                    need(8 + 4 * u)
                if pas == 2:
                    need(24 + 4 * u)
                groups = GROUPS[pas]
                per = -(-len(prev_work) // len(groups)) if prev_work else 0
                for grp in groups:
                    s_group(h, pas, grp, es_t)
                    for _ in range(per):
                        if prev_work:
                            prev_work.popleft()()
                    drip(2 if pas == 2 and h == 0 else 1)
                while prev_work:
                    prev_work.popleft()()

                def av_item(ib, h=h, pas=pas, es_t=es_t, y2t=y2t):
                    av_need(h, pas, ib)
                    av_ib(h, pas, ib, es_t, y2t)

                for ib in (range(8) if pas == 1 else range(8, 16)):
                    prev_work.append(lambda ib=ib, f=av_item: f(ib))
                if h % 2 == 1 and pas == 2:
                    prev_work.extend(
                        lambda u=u, pas=pas, r=r, y2t=y2t: transpose_one(
                            u, pas, r, y2t
                        )
                        for r in range(8)
                    )
                if pas == 1 and h == 0:
                    nc.sync.dma_start(
                        wp_sb[:], aps["wp"].rearrange("(l p) n -> p l n", p=P)
                    )

            if pas == 1:
                # pass-2-era fillers, appended before the pass-1 last pair so
                # its drips prefetch pair-0's tc>=2 qk chunks across the
                # pass boundary. Pass-1 transposes also defer to here.
                filler.extend(                                             # 20..35
                    lambda jq=jq, tci=tci: qk_chunk(jq, tci)
                    for jq in (0, 4, 1, 5, 2, 6, 3, 7) for tci in (2, 3)
                )
                filler.extend(                                             # 36..43
                    lambda jc=jc: v_chunk(jc) for jc in range(8, NJC)
                )
                filler.extend(                                             # 44..67
                    lambda u=u, r=r: transpose_one(u, 1, r, y2t_box[(1, u)])
                    for u in range(3) for r in range(8)
                )

            # ---- last pair (heads 6, 7), interleaved ----
            while prev_work:
                prev_work.popleft()()
            es6 = es_pool.tile([P, ES_COLS], bf16, tag="es", name=f"es{pas}_6")
            es7 = es_pool.tile([P, ES_COLS], bf16, tag="es", name=f"es{pas}_7")
            need(20 if pas == 1 else 36)
            y2t = y2_pool.tile([P, 8, P], bf16, tag="y2", name=f"y2{pas}_3")
            y2t_box[(pas, 3)] = y2t
            # pass 2 keeps single strips here: merged groups would bunch the
            # inline c_tiles at the very end of the kernel
            pair_groups = GROUPS[1] if pas == 1 else [(jc,) for jc in range(16)]
            done_jc, next_ib = set(), 0 if pas == 1 else 8
            for grp in pair_groups:
                s_group(6, pas, grp, es6)
                drip(1)
                s_group(7, pas, grp, es7)
                done_jc.update(grp)
                emitted = False
                while next_ib < njc and all(
                    jc in done_jc for jc in range(next_ib + 1)
                ):
                    ib, next_ib = next_ib, next_ib + 1
                    emitted = True
                    av_need(6, pas, ib)
                    av_ib(6, pas, ib, es6, y2t)
                    av_ib(7, pas, ib, es7, y2t)
                    if pas == 2:
                        transpose_one(3, pas, ib - 8, y2t)
                        c_tile(ib, 0)
                        c_tile(ib, 1)
                if not emitted:
                    drip(1)
            if pas == 1:
                filler.extend(                                             # 68..75
                    lambda r=r, y2t=y2t: transpose_one(3, 1, r, y2t)
                    for r in range(8)
                )
                filler.extend(                                             # 76..91
                    lambda t=t, o=o: c_tile(t, o)
                    for t in range(8) for o in range(2)
                )
        while filler:
            filler.popleft()()


_CACHE = {}


def build_nc():
    if "nc" in _CACHE:
        return _CACHE["nc"]
    nc = bacc.Bacc(
        "TRN2",
        target_bir_lowering=False,
        debug=False,
        enable_asserts=False,
        num_devices=8,
    )
    aps = {
        "xh": nc.dram_tensor("xh", [P, 4, 2, T], fp8, kind="ExternalInput").ap(),
        "xl": nc.dram_tensor("xl", [P, 4, 2, T], fp8, kind="ExternalInput").ap(),
        "wqh": nc.dram_tensor("wqh", [P, 8, 4, 2, P], fp8, kind="ExternalInput").ap(),
        "wql": nc.dram_tensor("wql", [P, 8, 4, 2, P], fp8, kind="ExternalInput").ap(),
        "wvh": nc.dram_tensor("wvh", [P, 4, 2, CL], fp8, kind="ExternalInput").ap(),
        "wvl": nc.dram_tensor("wvl", [P, 4, 2, CL], fp8, kind="ExternalInput").ap(),
        "bqk": nc.dram_tensor("bqk", [P, 8], f32, kind="ExternalInput").ap(),
        "bv": nc.dram_tensor("bv", [CL], f32, kind="ExternalInput").ap(),
        "wp": nc.dram_tensor("wp", [CL, C], bf16, kind="ExternalInput").ap(),
        "masks": nc.dram_tensor("masks", [P, P], bf16, kind="ExternalInput").ap(),
        "outp": nc.dram_tensor("outp", [T, C], f32, kind="ExternalOutput").ap(),
    }
    with tile.TileContext(nc) as tc:
        build_body(tc, aps)
    nc.compile()
    _CACHE["nc"] = nc
    return nc


F8NP = mybir.dt.np(fp8)


def _hi_lo(a):
    hi = a.astype(F8NP)
    lo = (a - hi.astype(np.float32)).astype(F8NP)
    return hi, lo


def _dr_layout(a, free_shape):
    # [C, N...] with contraction c = kc*256 + i*128 + p -> [128, 4, 2, N...]
    return np.ascontiguousarray(
        a.reshape(4, 2, P, *free_shape).transpose(2, 0, 1, 3)
    )


def make_in_maps(x, w_attn, b_attn, w_proj, b_proj):
    masks = np.triu(np.ones((P, P), dtype=np.float32)).astype(ml_dtypes.bfloat16)
    in_maps = []
    for core in range(8):
        b, g = core // 2, core % 2
        xT = np.ascontiguousarray(x[b].T)  # [C, T]
        xh, xl = _hi_lo(xT)
        qcols = slice(g * CL, (g + 1) * CL)
        kcols = slice(C + g * CL, C + (g + 1) * CL)
        vcols = slice(2 * C + g * CL, 2 * C + (g + 1) * CL)
        wqk = SC * np.concatenate([w_attn[:, qcols], w_attn[:, kcols]], axis=1)
        wqh, wql = _hi_lo(wqk)
        wvh, wvl = _hi_lo(SC * w_attn[:, vcols])
        bqk = SC * np.concatenate([b_attn[qcols], b_attn[kcols]]).reshape(8, P).T
        in_maps.append(
            {
                "xh": _dr_layout(xh, (T,)),
                "xl": _dr_layout(xl, (T,)),
                # [C, 1024] -> [4, 2, 128p, 8jq, 128m] -> [p, jq, kc, i, m]
                "wqh": np.ascontiguousarray(
                    wqh.reshape(4, 2, P, 8, P).transpose(2, 3, 0, 1, 4)
                ),
                "wql": np.ascontiguousarray(
                    wql.reshape(4, 2, P, 8, P).transpose(2, 3, 0, 1, 4)
                ),
                "wvh": _dr_layout(wvh, (CL,)),
                "wvl": _dr_layout(wvl, (CL,)),
                "bqk": np.ascontiguousarray(bqk),
                "bv": np.ascontiguousarray(SC * b_attn[vcols]),
                "wp": np.ascontiguousarray(
                    (w_proj[g * CL : (g + 1) * CL, :] / SC).astype(ml_dtypes.bfloat16)
                ),
                "masks": masks,
            }
        )
    return in_maps


def combine(parts, b_proj):
    return np.stack(
        [parts[2 * b] + parts[2 * b + 1] + b_proj[None, :] for b in range(B)]
    ).astype(np.float32)


def kernel(x, w_attn, b_attn, w_proj, b_proj, _trace=False, **run_kwargs):
    x = np.asarray(x, dtype=np.float32)
    w_attn = np.asarray(w_attn, dtype=np.float32)
    b_attn = np.asarray(b_attn, dtype=np.float32)
    w_proj = np.asarray(w_proj, dtype=np.float32)
    b_proj = np.asarray(b_proj, dtype=np.float32)

    nc = build_nc()
    in_maps = make_in_maps(x, w_attn, b_attn, w_proj, b_proj)
    try:
        res = bass_utils.run_bass_kernel_spmd(
            nc, in_maps, core_ids=list(range(8)), trace=_trace, **run_kwargs
        )
    except Exception:
        # transient NRT device wedge: one retry
        res = bass_utils.run_bass_kernel_spmd(
            nc, in_maps, core_ids=list(range(8)), trace=_trace, **run_kwargs
        )
    parts = [res.results[i]["outp"] for i in range(8)]
    out = combine(parts, b_proj)
    if _trace:
        return out, res
    return out


# revision 44
# speedup vs baseline: 1.1637x; 1.0079x over previous
"""Causal self-attention (B=4, T=2048, C=1024, H=16, D=64) on 8 trn2 cores.

Sharding: core i handles batch b = i//2 and head-group g = i%2 (8 of 16
heads), tensor-parallel over c_attn columns / c_proj rows. Each core
computes qkv for its heads, causal attention, and a partial projection
(its 512 rows of w_proj); the host sums the two partials per batch and
adds b_proj.

v2 pipeline (per core), built around the engine cost model:
  - q/k/v projections run on the PE in fp8 DoubleRow mode with a hi/lo
    split (x = x_hi + x_lo, w = w_hi + w_lo, three cross terms): 256-wide
    contraction per instruction at 0.5 cyc/col = 2.67x bf16 throughput
    at ~bf16 accuracy.
  - q/k are stored to SBUF as fp8 (e4m3) in a [64d, 2, T] pair-plane
    layout whose second plane is zeroed; S^T strips then also run in
    DoubleRow mode (2 cyc per 4 cols) despite the 64-deep contraction.
  - exp on ACT (the critical engine: ~139k cols x 0.83ns) writes bf16
    es strips; the causal diagonal block is masked by a DVE multiply.
  - AV is token-major: es strip blocks [128j, 128i] are the *stationary*
    operand, v_aug [128j, 65] (ones column -> denominators) the moving
    one, so each block costs 65 cycles and the softmax normalization
    becomes a per-partition reciprocal+scale fused into the PSUM drain.
  - y (token-major) is transposed back per 128x128 block on the PE and
    the projection runs as in the baseline (yT stationary, wp moving).
  - two passes over query halves (i < 1024, i >= 1024) bound es SBUF and
    let first-half projection tiles overlap second-half attention.
  - Engine budget: PE ~154us, ACT ~152us, DVE ~85us, Pool ~50us.
"""

import sys

sys.path.insert(0, "/opt/trn_rl_repo")

from collections import deque
from contextlib import ExitStack

import ml_dtypes
import numpy as np

import concourse.bass as bass
import concourse.mybir as mybir
import concourse.tile as tile
from concourse import bacc
from concourse import bass_utils

f32 = mybir.dt.float32
f32r = mybir.dt.float32r
bf16 = mybir.dt.bfloat16
fp8 = mybir.dt.float8e4
u32 = mybir.dt.uint32
EXP = mybir.ActivationFunctionType.Exp
MUL = mybir.AluOpType.mult
ADD = mybir.AluOpType.add
DR = mybir.MatmulPerfMode.DoubleRow

B, T, C, H, D = 4, 2048, 1024, 16, 64
HL = H // 2          # 8 heads per core
CL = HL * D          # 512 local feature width
P = 128
NJC = T // P         # 16 token chunks of 128

# Weights are pre-scaled by SC on the host so their hi/lo fp8 split stays in
# e4m3's normal range (w ~ N(0, 0.02^2) would otherwise land in subnormals
# where the lo residual quantizes to zero). q/k/v come out SC x too large;
# 1/SC^2 folds into the exp scale and 1/SC into w_proj.
SC = 32.0

# Strip widths: pass 1 covers i in [128jc, 1024), pass 2 covers
# i in [max(128jc, 1024), 2048).
W1 = [1024 - 128 * jc for jc in range(8)]
W2 = [min(1024, 2048 - 128 * jc) for jc in range(16)]

# Strips are packed into [128, 1024] PSUM tiles in complementary pairs so
# one exp instruction covers both (the second member accumulates onto the
# pending-zeroed second bank with start=False; its first 512+ columns of
# the pair always mark that bank first). GROUPS[pas] = list of jc-tuples.
GROUPS = {
    1: [(0,), (1, 7), (2, 6), (3, 5), (4,)],
    2: [(jc,) for jc in range(8)] + [(8,), (9, 15), (10, 14), (11, 13), (12,)],
}
ES_OFF = {1: {}, 2: {}}
for pas, groups in GROUPS.items():
    w = W1 if pas == 1 else W2
    off = 0
    for grp in groups:
        for jc in grp:
            ES_OFF[pas][jc] = off
            off += w[jc]
ES_COLS = max(ES_OFF[2][jc] + W2[jc] for jc in range(16))  # 12800


def build_body(tc, aps):
    nc = tc.nc

    with ExitStack() as ctx:
        const = ctx.enter_context(tc.tile_pool(name="const", bufs=1))
        xq_pool = ctx.enter_context(tc.tile_pool(name="xq", bufs=1))
        wq_pool = ctx.enter_context(tc.tile_pool(name="wq", bufs=1))
        wv_pool = ctx.enter_context(tc.tile_pool(name="wv", bufs=1))
        qk8_pool = ctx.enter_context(tc.tile_pool(name="qk8", bufs=1))
        vaug_pool = ctx.enter_context(tc.tile_pool(name="vaug", bufs=1))
        es_pool = ctx.enter_context(tc.tile_pool(name="es", bufs=2))
        y2_pool = ctx.enter_context(tc.tile_pool(name="y2", bufs=6))
        yT_pool = ctx.enter_context(tc.tile_pool(name="yT", bufs=1))
        wp_pool = ctx.enter_context(tc.tile_pool(name="wp", bufs=1))
        ostg = ctx.enter_context(tc.tile_pool(name="ostg", bufs=3))
        spill = ctx.enter_context(tc.tile_pool(name="spill", bufs=8))
        rc_pool = ctx.enter_context(tc.tile_pool(name="rc", bufs=2))
        psA = ctx.enter_context(tc.tile_pool(name="psA", bufs=2, space="PSUM"))
        psS = ctx.enter_context(tc.tile_pool(name="psS", bufs=2, space="PSUM"))
        psV = ctx.enter_context(tc.tile_pool(name="psV", bufs=2, space="PSUM"))

        # startup DMAs, kc-split so the first qk chunk's matmuls chase the
        # DMAs: SP runs pair-0 wq slices then xh halves, ACT (idle until its
        # first exp anyway) takes bqk + the other xh half, Pool takes xl.
        bqk_sb = const.tile([P, 8], f32)
        nc.scalar.dma_start(bqk_sb[:], aps["bqk"][:])
        wqh_sb = wq_pool.tile([P, 8, 4, 2, P], fp8, name="wqh_sb")
        wql_sb = wq_pool.tile([P, 8, 4, 2, P], fp8, name="wql_sb")
        for jq in (4, 0):
            nc.sync.dma_start(wqh_sb[:, jq], aps["wqh"][:, jq])
            nc.sync.dma_start(wql_sb[:, jq], aps["wql"][:, jq])
        xh_sb = xq_pool.tile([P, 4, 2, T], fp8, name="xh_sb")
        xl_sb = xq_pool.tile([P, 4, 2, T], fp8, name="xl_sb")
        for kc in range(4):
            (nc.sync if kc < 2 else nc.scalar).dma_start(
                xh_sb[:, kc], aps["xh"][:, kc]
            )
            nc.gpsimd.dma_start(xl_sb[:, kc], aps["xl"][:, kc])
        for jq in (1, 5, 2, 6, 3, 7):
            nc.gpsimd.dma_start(wqh_sb[:, jq], aps["wqh"][:, jq])
            nc.gpsimd.dma_start(wql_sb[:, jq], aps["wql"][:, jq])
        masks_sb = const.tile([P, P], bf16)
        nc.sync.dma_start(masks_sb[:], aps["masks"][:])
        wvh_sb = wv_pool.tile([P, 4, 2, CL], fp8, name="wvh_sb")
        wvl_sb = wv_pool.tile([P, 4, 2, CL], fp8, name="wvl_sb")
        nc.sync.dma_start(wvh_sb[:], aps["wvh"][:])
        nc.sync.dma_start(wvl_sb[:], aps["wvl"][:])
        bv_rep = const.tile([P, CL], f32)
        nc.sync.dma_start(bv_rep[:], aps["bv"][None, :].to_broadcast([P, CL]))

        # q/k fp8 pair-plane tiles, one per head-pair u: plane 0 = data,
        # plane 1 = zeros (kills the second DoubleRow term at 64-deep K).
        # Memsets go on DVE: the Pool queue is busy with the xl/wq DMAs.
        q8 = [qk8_pool.tile([P, 2, T], fp8, name=f"q8_{u}") for u in range(4)]
        k8 = [qk8_pool.tile([P, 2, T], fp8, name=f"k8_{u}") for u in range(4)]
        for t in k8 + q8:
            nc.vector.memset(t[:, 1, :].bitcast(u32), 0)

        vaug = vaug_pool.tile([P, NJC, HL, D + 1], bf16)
        nc.vector.memset(vaug[:, :, :, D : D + 1], 1.0)

        yT = yT_pool.tile([P, 4, T], bf16)
        wp_sb = wp_pool.tile([P, 4, C], bf16, name="wp_sb")

        # ---------------- work-unit emitters ----------------
        def qk_chunk(jq, tci):
            # 512 tokens of q (jq<4) or k (jq>=4) chunk -> fp8 store
            ps = psA.tile([P, 512], f32, tag="a")
            first = True
            for kc in range(4):
                for wsb, xsb in ((wqh_sb, xh_sb), (wqh_sb, xl_sb),
                                 (wql_sb, xh_sb)):
                    nc.tensor.matmul(
                        ps[:], wsb[:, jq, kc], xsb[:, kc, :, tci * 512 : tci * 512 + 512],
                        start=first, stop=(kc == 3 and wsb is wql_sb),
                        perf_mode=DR,
                    )
                    first = False
            dest = q8[jq] if jq < 4 else k8[jq - 4]
            nc.vector.tensor_scalar_add(
                dest[:, 0, tci * 512 : tci * 512 + 512], ps[:],
                bqk_sb[:, jq : jq + 1],
            )

        def v_chunk(jc):
            # 128 tokens of v for all 8 heads -> vaug bf16
            ps = psA.tile([P, 512], f32, tag="a")
            first = True
            for kc in range(4):
                for wsb, xsb in ((wvh_sb, xh_sb), (wvh_sb, xl_sb),
                                 (wvl_sb, xh_sb)):
                    nc.tensor.matmul(
                        ps[:], xsb[:, kc, :, jc * P : (jc + 1) * P],
                        wsb[:, kc],
                        start=first, stop=(kc == 3 and wsb is wvl_sb),
                        perf_mode=DR,
                    )
                    first = False
            nc.vector.tensor_tensor(
                vaug[:, jc, :, 0:D],
                ps[:].rearrange("p (h d) -> p h d", h=HL),
                bv_rep[:].rearrange("p (h d) -> p h d", h=HL), ADD,
            )

        def s_group(h, pas, grp, es_t):
            # one PSUM tile + one exp for a complementary pair of strips
            u, ko = h // 2, 64 * (h % 2)
            poff, wtot, es0 = 0, 0, ES_OFF[pas][grp[0]]
            for mi, jc in enumerate(grp):
                i0 = 128 * jc if pas == 1 else max(128 * jc, 1024)
                w = (1024 if pas == 1 else 2048) - i0
                wtot += w
            ps = psS.tile([P, 1024], f32, tag="s")
            for mi, jc in enumerate(grp):
                i0 = 128 * jc if pas == 1 else max(128 * jc, 1024)
                w = (1024 if pas == 1 else 2048) - i0
                for c0 in range(0, w, 512):
                    n = min(512, w - c0)
                    nc.tensor.matmul(
                        ps[:, poff + c0 : poff + c0 + n],
                        k8[u][ko : ko + 64, :, jc * P : (jc + 1) * P],
                        q8[u][ko : ko + 64, :, i0 + c0 : i0 + c0 + n],
                        start=(mi == 0), stop=True, perf_mode=DR,
                        skip_group_check=(mi > 0),
                    )
                poff += w
            nc.scalar.activation(
                es_t[:, es0 : es0 + wtot], ps[:, 0:wtot], EXP,
                scale=0.125 / (SC * SC),
            )
            for jc in grp:
                if pas == 1 or jc >= 8:
                    off = ES_OFF[pas][jc]
                    nc.gpsimd.tensor_tensor(
                        es_t[:, off : off + P], es_t[:, off : off + P],
                        masks_sb[:], MUL,
                    )

        def av_ib(h, pas, ib, es_t, y2t):
            ps = psV.tile([P, 512], f32, tag="av")
            for jc in range(ib + 1):
                if pas == 1:
                    col = ES_OFF[1][jc] + (ib - jc) * P
                else:
                    col = ES_OFF[2][jc] + ib * P - max(128 * jc, 1024)
                nc.tensor.matmul(
                    ps[:, 0 : D + 1],
                    es_t[:, col : col + P],
                    vaug[:, jc, h, :],
                    start=(jc == 0), stop=(jc == ib),
                )
            rc = rc_pool.tile([P, 1], f32, tag="rc")
            nc.vector.reciprocal(rc[:], ps[:, D : D + 1])
            nc.vector.tensor_scalar_mul(
                y2t[:, ib % 8, 64 * (h % 2) : 64 * (h % 2) + 64], ps[:, 0:D], rc[:]
            )

        def transpose_one(u, pas, r, y2t):
            # XBAR DMA transpose: off the PE/DVE critical engines entirely
            base = 0 if pas == 1 else 8
            nc.sync.dma_start_transpose(
                yT[:, u, (base + r) * P : (base + r + 1) * P], y2t[:, r, :]
            )

        def c_tile(tcb, oc):
            ps = psA.tile([P, 512], f32, tag="a")
            for lc in range(4):
                nc.tensor.matmul(
                    ps[:],
                    yT[:, lc, tcb * P : (tcb + 1) * P],
                    wp_sb[:, lc, oc * 512 : oc * 512 + 512],
                    start=(lc == 0), stop=(lc == 3),
                )
            ot = ostg.tile([P, 512], f32, tag="o")
            nc.vector.tensor_copy(ot[:], ps[:])
            nc.sync.dma_start(
                aps["outp"][tcb * P : (tcb + 1) * P, oc * 512 : oc * 512 + 512],
                ot[:],
            )

        # tail c_tiles (tcb 12..15) split: head-pairs 0-2 accumulate early and
        # spill to bf16; the pair-3-dependent lc=3 term finishes the tile.
        spill_box = {}

        def c_part(tcb, oc):
            ps = psA.tile([P, 512], f32, tag="a")
            for lc in range(3):
                nc.tensor.matmul(
                    ps[:],
                    yT[:, lc, tcb * P : (tcb + 1) * P],
                    wp_sb[:, lc, oc * 512 : oc * 512 + 512],
                    start=(lc == 0), stop=(lc == 2),
                )
            sp = spill.tile([P, 512], bf16, tag="sp", name=f"sp{tcb}_{oc}")
            spill_box[(tcb, oc)] = sp
            nc.vector.tensor_copy(sp[:], ps[:])

        def c_fin(tcb, oc):
            ps = psA.tile([P, 512], f32, tag="a")
            nc.tensor.matmul(
                ps[:],
                yT[:, 3, tcb * P : (tcb + 1) * P],
                wp_sb[:, 3, oc * 512 : oc * 512 + 512],
                start=True, stop=True,
            )
            ot = ostg.tile([P, 512], f32, tag="o")
            nc.vector.tensor_tensor(ot[:], ps[:], spill_box[(tcb, oc)][:], ADD)
            nc.sync.dma_start(
                aps["outp"][tcb * P : (tcb + 1) * P, oc * 512 : oc * 512 + 512],
                ot[:],
            )

        # ---------------- schedule ----------------
        filler = deque()
        state = {"done": 0}

        def need(k):
            while filler and state["done"] < k:
                filler.popleft()()
                state["done"] += 1

        def drip(n=1):
            for _ in range(n):
                if filler:
                    filler.popleft()()
                    state["done"] += 1

        # Pass 1 only ever reads q/k token-halves 0 and 1 (i < 1024), so only
        # those chunks run during pass 1 (pair 0 directly, pairs 1-3 via
        # fillers); every tc>=2 chunk, v chunks 8..15, the pass-1 transposes
        # and the first-half c_tiles all shift into pass 2, where the PE
        # otherwise idles under ACT's longer exp stream.
        qk_chunk(4, 0)
        qk_chunk(0, 0)
        qk_chunk(0, 1)
        qk_chunk(4, 1)
        filler.extend(lambda jc=jc: v_chunk(jc) for jc in range(8))        # 0..7
        for grp in ((1, 5), (2, 6), (3, 7)):                               # 8..19
            filler.extend(
                lambda jq=jq, tci=tci: qk_chunk(jq, tci)
                for jq in grp for tci in range(2)
            )

        # Heads 0..5 are software-pipelined: head h's AV/normalize/transpose
        # work (prev_work) executes interleaved into head h+1's strip loop so
        # the ACT exp stream never waits on a post-strip block. The last PAIR
        # (heads 6,7) interleaves both heads' strips and emits AV (plus
        # transposes/c_tiles in pass 2) as soon as each i-block completes,
        # spreading the would-be tail over the pair's whole exp stream.
        prev_work = deque()
        y2t_box = {}

        def av_need(h, pas, ib):
            if pas == 1:
                need(min(ib, 7) + 1 if h == 0 else 8)
            else:
                need(44)

        for pas in (1, 2):
            njc = 8 if pas == 1 else 16
            for h in range(HL - 2):
                u = h // 2
                es_t = es_pool.tile([P, ES_COLS], bf16, tag="es", name=f"es{pas}_{h}")
                if h % 2 == 0:
                    y2t_box[(pas, u)] = y2_pool.tile(
                        [P, 8, P], bf16, tag="y2", name=f"y2{pas}_{u}"
                    )
                y2t = y2t_box[(pas, u)]
                if pas == 1 and u > 0:
                    need(8 + 4 * u)
                if pas == 2:
                    need(24 + 4 * u)
                groups = GROUPS[pas]
                per = -(-len(prev_work) // len(groups)) if prev_work else 0
                for grp in groups:
                    s_group(h, pas, grp, es_t)
                    for _ in range(per):
                        if prev_work:
                            prev_work.popleft()()
                    drip(2 if pas == 2 and h == 0 else 1)
                while prev_work:
                    prev_work.popleft()()

                def av_item(ib, h=h, pas=pas, es_t=es_t, y2t=y2t):
                    av_need(h, pas, ib)
                    av_ib(h, pas, ib, es_t, y2t)

                for ib in (range(8) if pas == 1 else range(8, 16)):
                    prev_work.append(lambda ib=ib, f=av_item: f(ib))
                if h % 2 == 1 and pas == 2:
                    prev_work.extend(
                        lambda u=u, pas=pas, r=r, y2t=y2t: transpose_one(
                            u, pas, r, y2t
                        )
                        for r in range(8)
                    )
                if pas == 1 and h == 0:
                    nc.sync.dma_start(
                        wp_sb[:], aps["wp"].rearrange("(l p) n -> p l n", p=P)
                    )

            if pas == 1:
                # pass-2-era fillers, appended before the pass-1 last pair so
                # its drips prefetch pair-0's tc>=2 qk chunks across the
                # pass boundary. Pass-1 transposes also defer to here.
                filler.extend(                                             # 20..35
                    lambda jq=jq, tci=tci: qk_chunk(jq, tci)
                    for jq in (0, 4, 1, 5, 2, 6, 3, 7) for tci in (2, 3)
                )
                filler.extend(                                             # 36..43
                    lambda jc=jc: v_chunk(jc) for jc in range(8, NJC)
                )
                filler.extend(                                             # 44..67
                    lambda u=u, r=r: transpose_one(u, 1, r, y2t_box[(1, u)])
                    for u in range(3) for r in range(8)
                )

            # ---- last pair (heads 6, 7), interleaved ----
            while prev_work:
                prev_work.popleft()()
            es6 = es_pool.tile([P, ES_COLS], bf16, tag="es", name=f"es{pas}_6")
            es7 = es_pool.tile([P, ES_COLS], bf16, tag="es", name=f"es{pas}_7")
            need(20 if pas == 1 else 36)
            y2t = y2_pool.tile([P, 8, P], bf16, tag="y2", name=f"y2{pas}_3")
            y2t_box[(pas, 3)] = y2t
            # pass 2 keeps single strips here: merged groups would bunch the
            # inline c_tiles at the very end of the kernel
            pair_groups = GROUPS[1] if pas == 1 else [(jc,) for jc in range(16)]
            done_jc, next_ib = set(), 0 if pas == 1 else 8
            for gi, grp in enumerate(pair_groups):
                s_group(6, pas, grp, es6)
                drip(1)
                s_group(7, pas, grp, es7)
                done_jc.update(grp)
                emitted = False
                while next_ib < njc and all(
                    jc in done_jc for jc in range(next_ib + 1)
                ):
                    ib, next_ib = next_ib, next_ib + 1
                    emitted = True
                    av_need(6, pas, ib)
                    av_ib(6, pas, ib, es6, y2t)
                    av_ib(7, pas, ib, es7, y2t)
                    if pas == 2:
                        transpose_one(3, pas, ib - 8, y2t)
                        if ib < 12:
                            c_tile(ib, 0)
                            c_tile(ib, 1)
                        else:
                            c_fin(ib, 0)
                            c_fin(ib, 1)
                if pas == 2 and gi < 8:
                    # pre-accumulate the pair-0..2 part of the tail c_tiles
                    c_part(12 + gi // 2, gi % 2)
                elif not emitted:
                    drip(1)
            if pas == 1:
                filler.extend(                                             # 68..75
                    lambda r=r, y2t=y2t: transpose_one(3, 1, r, y2t)
                    for r in range(8)
                )
                filler.extend(                                             # 76..91
                    lambda t=t, o=o: c_tile(t, o)
                    for t in range(8) for o in range(2)
                )
        while filler:
            filler.popleft()()


_CACHE = {}


def build_nc():
    if "nc" in _CACHE:
        return _CACHE["nc"]
    nc = bacc.Bacc(
        "TRN2",
        target_bir_lowering=False,
        debug=False,
        enable_asserts=False,
        num_devices=8,
    )
    aps = {
        "xh": nc.dram_tensor("xh", [P, 4, 2, T], fp8, kind="ExternalInput").ap(),
        "xl": nc.dram_tensor("xl", [P, 4, 2, T], fp8, kind="ExternalInput").ap(),
        "wqh": nc.dram_tensor("wqh", [P, 8, 4, 2, P], fp8, kind="ExternalInput").ap(),
        "wql": nc.dram_tensor("wql", [P, 8, 4, 2, P], fp8, kind="ExternalInput").ap(),
        "wvh": nc.dram_tensor("wvh", [P, 4, 2, CL], fp8, kind="ExternalInput").ap(),
        "wvl": nc.dram_tensor("wvl", [P, 4, 2, CL], fp8, kind="ExternalInput").ap(),
        "bqk": nc.dram_tensor("bqk", [P, 8], f32, kind="ExternalInput").ap(),
        "bv": nc.dram_tensor("bv", [CL], f32, kind="ExternalInput").ap(),
        "wp": nc.dram_tensor("wp", [CL, C], bf16, kind="ExternalInput").ap(),
        "masks": nc.dram_tensor("masks", [P, P], bf16, kind="ExternalInput").ap(),
        "outp": nc.dram_tensor("outp", [T, C], f32, kind="ExternalOutput").ap(),
    }
    with tile.TileContext(nc) as tc:
        build_body(tc, aps)
    nc.compile()
    _CACHE["nc"] = nc
    return nc


F8NP = mybir.dt.np(fp8)


def _hi_lo(a):
    hi = a.astype(F8NP)
    lo = (a - hi.astype(np.float32)).astype(F8NP)
    return hi, lo


def _dr_layout(a, free_shape):
    # [C, N...] with contraction c = kc*256 + i*128 + p -> [128, 4, 2, N...]
    return np.ascontiguousarray(
        a.reshape(4, 2, P, *free_shape).transpose(2, 0, 1, 3)
    )


def make_in_maps(x, w_attn, b_attn, w_proj, b_proj):
    masks = np.triu(np.ones((P, P), dtype=np.float32)).astype(ml_dtypes.bfloat16)
    in_maps = []
    for core in range(8):
        b, g = core // 2, core % 2
        xT = np.ascontiguousarray(x[b].T)  # [C, T]
        xh, xl = _hi_lo(xT)
        qcols = slice(g * CL, (g + 1) * CL)
        kcols = slice(C + g * CL, C + (g + 1) * CL)
        vcols = slice(2 * C + g * CL, 2 * C + (g + 1) * CL)
        wqk = SC * np.concatenate([w_attn[:, qcols], w_attn[:, kcols]], axis=1)
        wqh, wql = _hi_lo(wqk)
        wvh, wvl = _hi_lo(SC * w_attn[:, vcols])
        bqk = SC * np.concatenate([b_attn[qcols], b_attn[kcols]]).reshape(8, P).T
        in_maps.append(
            {
                "xh": _dr_layout(xh, (T,)),
                "xl": _dr_layout(xl, (T,)),
                # [C, 1024] -> [4, 2, 128p, 8jq, 128m] -> [p, jq, kc, i, m]
                "wqh": np.ascontiguousarray(
                    wqh.reshape(4, 2, P, 8, P).transpose(2, 3, 0, 1, 4)
                ),
                "wql": np.ascontiguousarray(
                    wql.reshape(4, 2, P, 8, P).transpose(2, 3, 0, 1, 4)
                ),
                "wvh": _dr_layout(wvh, (CL,)),
                "wvl": _dr_layout(wvl, (CL,)),
                "bqk": np.ascontiguousarray(bqk),
                "bv": np.ascontiguousarray(SC * b_attn[vcols]),
                "wp": np.ascontiguousarray(
                    (w_proj[g * CL : (g + 1) * CL, :] / SC).astype(ml_dtypes.bfloat16)
                ),
                "masks": masks,
            }
        )
    return in_maps


def combine(parts, b_proj):
    return np.stack(
        [parts[2 * b] + parts[2 * b + 1] + b_proj[None, :] for b in range(B)]
    ).astype(np.float32)


def kernel(x, w_attn, b_attn, w_proj, b_proj, _trace=False, **run_kwargs):
    x = np.asarray(x, dtype=np.float32)
    w_attn = np.asarray(w_attn, dtype=np.float32)
    b_attn = np.asarray(b_attn, dtype=np.float32)
    w_proj = np.asarray(w_proj, dtype=np.float32)
    b_proj = np.asarray(b_proj, dtype=np.float32)

    nc = build_nc()
    in_maps = make_in_maps(x, w_attn, b_attn, w_proj, b_proj)
    try:
        res = bass_utils.run_bass_kernel_spmd(
            nc, in_maps, core_ids=list(range(8)), trace=_trace, **run_kwargs
        )
    except Exception:
        # transient NRT device wedge: one retry
        res = bass_utils.run_bass_kernel_spmd(
            nc, in_maps, core_ids=list(range(8)), trace=_trace, **run_kwargs
        )
    parts = [res.results[i]["outp"] for i in range(8)]
    out = combine(parts, b_proj)
    if _trace:
        return out, res
    return out
